# revision 24
# baseline (speedup 1.0000x reference)
"""Bass/Trainium2 kernel for nn_EnhancedContrast (8-core SPMD).

Sharding: rows (N=8192) split across 8 cores, 1024 rows each. Each core:
  - runs the projection MLP on its za/zb row-slab (activation-transposed layout,
    zb first so the single all-gather of normalized zb overlaps za's MLP),
  - computes its row-slab of m = exp(cos/tau): rowsum/dot_a reduce locally along
    the free dim; colsum partials ride a ones-column on the one-hot matmul and
    dot_b partials come from a host-transposed bf16 pos^T via a ones-matmul
    partition reduction,
  - one AllReduce (~84KB) combines batch_sim/pos_sim_graph/colsum/dot_b/log
    partials; every core then computes the identical scalar loss.
"""
import sys

sys.path.insert(0, "/opt/trn_rl_repo")

import numpy as np

N, H, D, G = 8192, 512, 256, 64
TAU, LAM, ALPHA, BETA = 0.5, 0.5, 1.0, 1.0
EPS_G, EPS_L = 1e-6, 1e-5
NC = 8            # cores
R = N // NC       # rows per core = 1024
NB = R // 128     # 128-row blocks per core = 8
HC = H // 128     # hidden chunks = 4
DC = D // 128     # proj-dim chunks = 2
NS = R // 512     # 512-wide moving slices per 1024 = 2

LAST_RESULTS = None  # stashed BassKernelResults for test.py
_PROGRAM_CACHE = {}


def _build_program(batch_np):
    import concourse.mybir as mybir
    import concourse.tile as tile
    from concourse import bacc
    from concourse.masks import make_identity

    F32 = mybir.dt.float32
    F32R = mybir.dt.float32r
    BF16 = mybir.dt.bfloat16
    I32 = mybir.dt.int32
    AF = mybir.ActivationFunctionType
    ALU = mybir.AluOpType
    X = mybir.AxisListType.X

    # group segments along the full column axis (batch is sorted)
    bounds = np.searchsorted(batch_np, np.arange(G + 1))
    segs_by_jc = [[] for _ in range(NB)]
    for g in range(G):
        lo, hi = int(bounds[g]), int(bounds[g + 1])
        first = True
        j = lo
        while j < hi:
            jc = j // R
            e = min(hi, (jc + 1) * R)
            segs_by_jc[jc].append((g, j - jc * R, e - jc * R, first))
            first = False
            j = e

    nc = bacc.Bacc("TRN2", target_bir_lowering=False, debug=False, num_devices=NC)

    # ---- I/O ----
    za_d = nc.dram_tensor("za_s", [R, H], F32, kind="ExternalInput")
    zb_d = nc.dram_tensor("zb_s", [R, H], F32, kind="ExternalInput")
    pos_d = nc.dram_tensor("pos_s", [R, N], BF16, kind="ExternalInput")
    posT_d = nc.dram_tensor("posT_s", [R, N], BF16, kind="ExternalInput")
    hot_d = nc.dram_tensor("hot_s", [R, 65], BF16, kind="ExternalInput")
    w1_d = nc.dram_tensor("W1", [H, H], F32R, kind="ExternalInput")
    w2_d = nc.dram_tensor("W2", [H, H], F32R, kind="ExternalInput")
    w3_d = nc.dram_tensor("W3", [H, D], F32R, kind="ExternalInput")
    prm_d = nc.dram_tensor("prm", [128, 32], F32, kind="ExternalInput")
    coef_d = nc.dram_tensor("coef", [1, 8], F32, kind="ExternalInput")
    out_d = nc.dram_tensor("out_s", [R, 2 * D], F32, kind="ExternalOutput")
    loss_d = nc.dram_tensor("loss", [1, 1], F32, kind="ExternalOutput")

    with tile.TileContext(nc) as tc:
        with tc.tile_pool(name="consts", bufs=1) as consts, \
             tc.tile_pool(name="live", bufs=1) as live, \
             tc.tile_pool(name="dram", bufs=1, space="DRAM") as dram:
            # packed per-partition params: b1c g1c be1c b2c g2c be2c (4 cols each),
            # b3c (2), a1, a2, ln2 -> 29 cols used
            prm = consts.tile([128, 32], F32)
            nc.sync.dma_start(out=prm[:], in_=prm_d[:])
            b1c, g1c, be1c = prm[:, 0:4], prm[:, 4:8], prm[:, 8:12]
            b2c, g2c, be2c = prm[:, 12:16], prm[:, 16:20], prm[:, 20:24]
            b3c = prm[:, 24:26]
            a1_bc, a2_bc = prm[:, 26:27], prm[:, 27:28]
            ln2_t = prm[0:1, 28:29]
            coef_sb = consts.tile([1, 8], F32)
            nc.sync.dma_start(out=coef_sb[:], in_=coef_d[:])

            ident = consts.tile([128, 128], F32)
            make_identity(nc, ident[:])
            ones_k32 = consts.tile([128, 1], F32)
            nc.vector.memset(ones_k32[:], 1.0)
            ones_k = consts.tile([128, 1], F32R)
            nc.vector.tensor_copy(ones_k[:], ones_k32[:])
            ones_kb = consts.tile([128, 1], BF16)
            nc.vector.tensor_copy(ones_kb[:], ones_k32[:])
            ones2_32 = consts.tile([128, 2], F32)
            nc.vector.memset(ones2_32[:], 1.0)
            ones2 = consts.tile([128, 2], F32R)
            nc.vector.tensor_copy(ones2[:], ones2_32[:])

            ab1 = consts.tile([128, HC], F32)
            nc.vector.tensor_scalar(out=ab1[:], in0=b1c, scalar1=a1_bc, scalar2=None,
                                    op0=ALU.mult)
            ab2 = consts.tile([128, HC], F32)
            nc.vector.tensor_scalar(out=ab2[:], in0=b2c, scalar1=a2_bc, scalar2=None,
                                    op0=ALU.mult)

            # one-hot (64 groups + ones column) per row-block, host-prepared
            hot_r = consts.tile([128, NB, 65], BF16)
            nc.sync.dma_start(out=hot_r[:],
                              in_=hot_d[:].rearrange("(b p) g -> p b g", p=128))

            # eye64 for diag extraction in the epilogue
            iota_i = consts.tile([1, G], I32)
            nc.gpsimd.iota(iota_i[:], pattern=[[1, G]], base=0, channel_multiplier=0)
            iota_bc_i = consts.tile([128, G], I32)
            nc.gpsimd.partition_broadcast(iota_bc_i[:], iota_i[:])
            iota_f = consts.tile([128, G], F32)
            nc.vector.tensor_copy(iota_f[:], iota_bc_i[:])
            iota_col_i = consts.tile([128, 1], I32)
            nc.gpsimd.iota(iota_col_i[:], pattern=[[0, 1]], base=0, channel_multiplier=1)
            iota_col = consts.tile([128, 1], F32)
            nc.vector.tensor_copy(iota_col[:], iota_col_i[:])
            eye64 = consts.tile([G, G], F32)
            nc.vector.tensor_scalar(out=eye64[:], in0=iota_f[0:G, :],
                                    scalar1=iota_col[0:G, :], scalar2=None,
                                    op0=ALU.is_equal)

            # long-lived similarity-phase tensors
            za_r = live.tile([128, DC, R], BF16)     # stationary for the m-slab matmuls
            invna_cols = live.tile([128, NB], F32)
            invnb_cols = live.tile([128, NB], F32)
            rsacc = live.tile([128, NB * NB], F32)   # rowsum partials, col = ib*NB+jc
            daacc = live.tile([128, NB * NB], F32)
            bs_sb = live.tile([G, G], F32)
            nc.vector.memset(bs_sb[:], 0.0)
            psg_sb = live.tile([G, 1], F32)
            v01_sb = live.tile([1, 2], F32)
            ps_cols_r = live.tile([128, NB, 2], BF16)

            # collective buffers
            ag_in = dram.tile([DC, 128, R], BF16)
            ag_out = dram.tile([NC, DC, 128, R], BF16, addr_space="Shared")
            o1, o2, o3, o4 = G * G, G * G + G, G * G + G + 2, G * G + G + 2 + N
            CC2 = o4 + N
            cc2_in = dram.tile([CC2], F32)
            cc2_out = dram.tile([CC2], F32, addr_space="Shared")

            # ------------- Phase A: MLP (zb first; its gather overlaps za) -------------
            with tc.tile_pool(name="wpool", bufs=1) as wp, \
                 tc.tile_pool(name="mlp", bufs=1) as mp, \
                 tc.tile_pool(name="mps", bufs=2, space="PSUM") as mpp, \
                 tc.tile_pool(name="tps", bufs=2, space="PSUM") as tpp, \
                 tc.tile_pool(name="stg", bufs=1) as sg, \
                 tc.tile_pool(name="npsn", bufs=2, space="PSUM") as npn:
                w1 = wp.tile([128, HC, H], F32R)
                nc.sync.dma_start(out=w1[:], in_=w1_d[:].rearrange("(c p) o -> p c o", p=128))
                w2 = wp.tile([128, HC, H], F32R)
                nc.sync.dma_start(out=w2[:], in_=w2_d[:].rearrange("(c p) o -> p c o", p=128))
                w3 = wp.tile([128, HC, D], F32R)
                nc.sync.dma_start(out=w3[:], in_=w3_d[:].rearrange("(c p) o -> p c o", p=128))

                def load_xT(side, x_d):
                    xT = mp.tile([128, HC, R], F32R, tag="big", bufs=3, name=f"xT{side}")
                    for rb in range(NB):
                        st = sg.tile([128, H], F32, tag="stage", bufs=3)
                        nc.sync.dma_start(out=st[:], in_=x_d[rb * 128:(rb + 1) * 128, :])
                        tp4 = tpp.tile([128, H], F32, tag="tp")
                        for c in range(HC):
                            nc.tensor.transpose(tp4[:, c * 128:(c + 1) * 128],
                                                st[:, c * 128:(c + 1) * 128], ident[:])
                        nc.vector.tensor_copy(
                            xT[:, :, rb * 128:(rb + 1) * 128],
                            tp4[:].rearrange("p (c q) -> p c q", c=HC))
                    return xT

                def layer(x_in, w, n_ob, act_fn, scale, bias_cols, gc, bec, out_tag,
                          out_dtype):
                    bufs = 3 if out_tag == "big" else 1
                    out_t = mp.tile([128, n_ob, R], out_dtype, tag=out_tag, bufs=bufs,
                                    name=f"L{out_tag}{n_ob}")
                    for ob in range(n_ob):
                        ps = mpp.tile([128, R], F32, tag="mm")
                        for ns in range(NS):
                            sl = slice(ns * 512, (ns + 1) * 512)
                            for c in range(HC):
                                nc.tensor.matmul(ps[:, sl],
                                                 w[:, c, ob * 128:(ob + 1) * 128],
                                                 x_in[:, c, sl], start=(c == 0),
                                                 stop=(c == HC - 1))
                        if gc is not None:
                            th = sg.tile([128, R], F32, tag="th", bufs=2)
                            nc.scalar.activation(th[:], ps[:], act_fn, scale=scale,
                                                 bias=bias_cols[:, ob:ob + 1])
                            nc.vector.tensor_scalar(out=out_t[:, ob, :], in0=th[:],
                                                    scalar1=gc[:, ob:ob + 1],
                                                    scalar2=bec[:, ob:ob + 1],
                                                    op0=ALU.mult, op1=ALU.add)
                        else:
                            nc.scalar.activation(out_t[:, ob, :], ps[:], act_fn,
                                                 scale=scale,
                                                 bias=bias_cols[:, ob:ob + 1])
                    return out_t

                zpT = {}
                for side, x_d in (("b", zb_d), ("a", za_d)):
                    xT = load_xT(side, x_d)
                    h1 = layer(xT, w1, HC, AF.Tanh, a1_bc, ab1, g1c, be1c, "big", F32R)
                    h2 = layer(h1, w2, HC, AF.Tanh, a2_bc, ab2, g2c, be2c, "big", F32R)
                    zpT[side] = layer(h2, w3, DC, AF.Silu, 1.0, b3c, None, None,
                                      f"zp{side}", F32)

                    sq = sg.tile([128, DC, R], F32R, tag="sq")
                    nc.vector.tensor_mul(sq[:], zpT[side][:], zpT[side][:])
                    lnr = sg.tile([1, R], F32, tag="lnr")
                    for ns in range(NS):
                        sl = slice(ns * 512, (ns + 1) * 512)
                        ns2 = npn.tile([1, 512], F32, tag="smallps", name="ns2")
                        for c in range(DC):
                            nc.tensor.matmul(ns2[:], ones_k[:], sq[:, c, sl],
                                             start=(c == 0), stop=(c == DC - 1))
                        nc.scalar.activation(lnr[:, sl], ns2[:], AF.Ln)
                    if side == "b":
                        # zb_hat = zb_pT * (invnb/tau)[r]; gather it (the only gather)
                        invnt = sg.tile([1, R], F32, tag="invnt")
                        nc.scalar.activation(invnt[:], lnr[:], AF.Exp, scale=-0.5,
                                             bias=ln2_t)
                        bc = sg.tile([128, R], F32, tag="bc")
                        nc.gpsimd.partition_broadcast(bc[:], invnt[:])
                        hat = sg.tile([128, DC, R], BF16, tag="hat")
                        for c in range(DC):
                            nc.vector.tensor_mul(hat[:, c, :], zpT[side][:, c, :], bc[:])
                        nc.sync.dma_start(out=ag_in[:].rearrange("c p r -> p c r"),
                                          in_=hat[:])
                        nc.gpsimd.collective_compute(
                            "AllGather", ALU.bypass, replica_groups=[list(range(NC))],
                            ins=[ag_in[:].opt()], outs=[ag_out[:].opt()])
                        # invnb cols (for the pos_sim diag path): invnb = invnt/2
                        dnormb = dram.tile([1, R], F32, tag="dnormb")
                        nc.sync.dma_start(out=dnormb[:], in_=invnt[:])
                        ivb = sg.tile([128, NB], F32, tag="ivb")
                        nc.sync.dma_start(
                            out=ivb[:],
                            in_=dnormb[0:1, :].rearrange("o (b p) -> (o p) b", b=NB))
                        nc.vector.tensor_scalar(out=invnb_cols[:], in0=ivb[:],
                                                scalar1=float(TAU), scalar2=None,
                                                op0=ALU.mult)
                    else:
                        # za stays raw; invna applied at exp time (per-partition scale)
                        invn = sg.tile([1, R], F32, tag="invnt")
                        nc.scalar.activation(invn[:], lnr[:], AF.Exp, scale=-0.5)
                        dnorma = dram.tile([1, R], F32, tag="dnorma")
                        nc.sync.dma_start(out=dnorma[:], in_=invn[:])
                        nc.sync.dma_start(
                            out=invna_cols[:],
                            in_=dnorma[0:1, :].rearrange("o (b p) -> (o p) b", b=NB))

                nc.vector.tensor_copy(za_r[:], zpT["a"][:])

                # output slab: transpose zpT back to natural and store
                for rb in range(NB):
                    tp4 = tpp.tile([128, 2 * D], F32, tag="tp")
                    for k, side in enumerate(("a", "b")):
                        for ob in range(DC):
                            nc.tensor.transpose(
                                tp4[:, k * D + ob * 128:k * D + (ob + 1) * 128],
                                zpT[side][:, ob, rb * 128:(rb + 1) * 128], ident[:])
                    ost = sg.tile([128, 2 * D], F32, tag="ost", bufs=2)
                    nc.vector.tensor_copy(ost[:], tp4[:])
                    nc.sync.dma_start(out=out_d[rb * 128:(rb + 1) * 128, :], in_=ost[:])

                # pos_sim (diag of m): exp(rawdot*invna*invnb/tau); psg partial
                prod = sg.tile([128, DC, R], F32R, tag="sq")
                nc.vector.tensor_mul(prod[:], zpT["a"][:], zpT["b"][:])
                rd_ps = npn.tile([128, NB, 2], F32, tag="smallps", name="rd_ps")
                for ib in range(NB):
                    for c in range(DC):
                        nc.tensor.matmul(rd_ps[:, ib, :],
                                         prod[:, c, ib * 128:(ib + 1) * 128],
                                         ones2[:], start=(c == 0), stop=(c == DC - 1))
                t1 = sg.tile([128, NB], F32, tag="t1")
                nc.vector.tensor_mul(t1[:], rd_ps[:, :, 0], invna_cols[:])
                t2 = sg.tile([128, NB], F32, tag="t2")
                nc.vector.tensor_mul(t2[:], t1[:], invnb_cols[:])
                ps_cols = sg.tile([128, NB], F32, tag="t3")
                nc.scalar.activation(ps_cols[:], t2[:], AF.Exp, scale=float(1.0 / TAU))
                zcols = sg.tile([128, NB], F32, tag="zcols")
                nc.vector.memset(zcols[:], 0.0)
                nc.vector.tensor_copy(ps_cols_r[:, :, 1], zcols[:])
                nc.vector.tensor_copy(ps_cols_r[:, :, 0], ps_cols[:])
                psg_ps = npn.tile([G, 2], F32, tag="smallps", name="psg_ps")
                for ib in range(NB):
                    nc.tensor.matmul(psg_ps[:], hot_r[:, ib, 0:G], ps_cols_r[:, ib, :],
                                     start=(ib == 0), stop=(ib == NB - 1))
                nc.vector.tensor_copy(psg_sb[:], psg_ps[:, 0:1])

            # ---------------- Phase C: similarity slab ----------------
            with tc.tile_pool(name="strm", bufs=3) as strm, \
                 tc.tile_pool(name="pospool", bufs=3) as pp, \
                 tc.tile_pool(name="mpool", bufs=3) as mpo, \
                 tc.tile_pool(name="scr", bufs=2) as scp, \
                 tc.tile_pool(name="sps", bufs=2, space="PSUM") as sps, \
                 tc.tile_pool(name="ups", bufs=2, space="PSUM") as ups, \
                 tc.tile_pool(name="tiny", bufs=4) as tiny:
                for jc in range(NB):
                    zb_sl = strm.tile([128, DC, R], BF16, tag="zb_sl", name="zb_sl")
                    nc.sync.dma_start(out=zb_sl[:],
                                      in_=ag_out[jc].rearrange("c p r -> p c r"))
                    # u rows 0..64: [hot|ones]^T @ m  (row 64 = colsum partial);
                    # row 96: ones^T @ (m * posT) = dot_b partial
                    u_ps = ups.tile([128, R], F32, tag="u", name="u_ps")
                    for ib in range(NB):
                        ibs = slice(ib * 128, (ib + 1) * 128)
                        acol = ib * NB + jc
                        pos_t = pp.tile([128, R], BF16, tag="pos", name="pos_t")
                        nc.gpsimd.dma_start(out=pos_t[:],
                                            in_=pos_d[ibs, jc * R:(jc + 1) * R])
                        posT_t = pp.tile([128, R], BF16, tag="posT", name="posT_t")
                        nc.gpsimd.dma_start(out=posT_t[:],
                                            in_=posT_d[ibs, jc * R:(jc + 1) * R])
                        sa = sps.tile([128, R], F32, tag="S", name="sa")
                        for ns in range(NS):
                            sl = slice(ns * 512, (ns + 1) * 512)
                            for c in range(DC):
                                nc.tensor.matmul(sa[:, sl], za_r[:, c, ibs],
                                                 zb_sl[:, c, sl],
                                                 start=(c == 0), stop=(c == DC - 1))
                        m_a = mpo.tile([128, R], BF16, tag="ma", name="m_a")
                        nc.scalar.activation(m_a[:], sa[:], AF.Exp,
                                             scale=invna_cols[:, ib:ib + 1],
                                             accum_out=rsacc[:, acol:acol + 1])
                        scr_a = scp.tile([128, R], BF16, tag="scra", name="scr_a")
                        nc.vector.scalar_tensor_tensor(
                            out=scr_a[:], in0=m_a[:], scalar=1.0, in1=pos_t[:],
                            op0=ALU.mult, op1=ALU.mult,
                            accum_out=daacc[:, acol:acol + 1])
                        prod_b = scp.tile([128, R], BF16, tag="prodb", name="prod_b")
                        nc.vector.tensor_mul(prod_b[:], m_a[:], posT_t[:])
                        for ns in range(NS):
                            sl = slice(ns * 512, (ns + 1) * 512)
                            nc.tensor.matmul(u_ps[0:65, sl], hot_r[:, ib, :], m_a[:, sl],
                                             start=(ib == 0), stop=(ib == NB - 1),
                                             skip_group_check=True)
                            nc.tensor.matmul(u_ps[96:97, sl], ones_kb[:], prod_b[:, sl],
                                             start=(ib == 0), stop=(ib == NB - 1),
                                             skip_group_check=True,
                                             tile_position=(0, 96))
                    csdb_st = scp.tile([128, R], F32, tag="csdb", name="csdb_st",
                                       bufs=2)
                    nc.scalar.copy(csdb_st[64:65, :], u_ps[64:65, :])
                    nc.scalar.copy(csdb_st[96:97, :], u_ps[96:97, :])
                    nc.sync.dma_start(
                        out=cc2_in[o3 + jc * R:o3 + (jc + 1) * R].rearrange(
                            "(o f) -> o f", o=1),
                        in_=csdb_st[64:65, :])
                    nc.sync.dma_start(
                        out=cc2_in[o4 + jc * R:o4 + (jc + 1) * R].rearrange(
                            "(o f) -> o f", o=1),
                        in_=csdb_st[96:97, :])
                    for (g, lo, hi, first) in segs_by_jc[jc]:
                        if first:
                            nc.vector.reduce_sum(bs_sb[:, g:g + 1], u_ps[0:G, lo:hi],
                                                 axis=X)
                        else:
                            tmp = tiny.tile([G, 1], F32, tag="segtmp", name="segtmp")
                            nc.vector.reduce_sum(tmp[:], u_ps[0:G, lo:hi], axis=X)
                            nc.vector.tensor_add(bs_sb[:, g:g + 1], bs_sb[:, g:g + 1],
                                                 tmp[:])

            # ---------------- Phase D: local log-sums + allreduce ----------------
            with tc.tile_pool(name="ep", bufs=1) as ep, \
                 tc.tile_pool(name="eps", bufs=2, space="PSUM") as epp:
                red = ep.tile([128, 2, NB], F32)
                for k, acc in enumerate((daacc, rsacc)):
                    nc.vector.reduce_sum(red[:, k, :],
                                         acc[:].rearrange("p (ib jc) -> p ib jc", ib=NB),
                                         axis=X)
                nc.vector.tensor_scalar(out=red[:, 1, :], in0=red[:, 1, :],
                                        scalar1=EPS_G, scalar2=None, op0=ALU.add)
                la2 = ep.tile([128, 2], F32)
                lnscr = ep.tile([128, NB], F32)
                for k in range(2):
                    nc.scalar.activation(lnscr[:], red[:, k, :], AF.Ln,
                                         accum_out=la2[:, k:k + 1])
                la2r = ep.tile([128, 2], F32R)
                nc.vector.tensor_copy(la2r[:], la2[:])
                v01_ps = epp.tile([1, 2], F32)
                nc.tensor.matmul(v01_ps[:], ones_k[:], la2r[:], start=True, stop=True)
                nc.vector.tensor_copy(v01_sb[:], v01_ps[:])

                nc.sync.dma_start(out=cc2_in[0:o1].rearrange("(g h) -> g h", g=G),
                                  in_=bs_sb[:])
                nc.sync.dma_start(out=cc2_in[o1:o2].rearrange("(g o) -> g o", g=G),
                                  in_=psg_sb[:])
                nc.sync.dma_start(out=cc2_in[o2:o3].rearrange("(o f) -> o f", o=1),
                                  in_=v01_sb[:])
                nc.gpsimd.collective_compute(
                    "AllReduce", ALU.add, replica_groups=[list(range(NC))],
                    ins=[cc2_in[:].opt()], outs=[cc2_out[:].opt()])

                # ---------------- Phase E: final scalar loss ----------------
                bs_f = ep.tile([G, G], F32)
                nc.sync.dma_start(out=bs_f[:],
                                  in_=cc2_out[0:o1].rearrange("(g h) -> g h", g=G))
                psg_f = ep.tile([G, 1], F32)
                nc.sync.dma_start(out=psg_f[:],
                                  in_=cc2_out[o1:o2].rearrange("(g o) -> g o", g=G))
                # full colsum/dot_b rows -> [128, 64] col layout
                csdb = ep.tile([128, 2, G], F32)
                nc.sync.dma_start(out=csdb[:, 0, :],
                                  in_=cc2_out[o3:o4].rearrange("(b p) -> p b", p=128))
                nc.sync.dma_start(out=csdb[:, 1, :],
                                  in_=cc2_out[o4:].rearrange("(b p) -> p b", p=128))
                # v2 = sum ln(dot_b), v3 = sum ln(colsum+eps) over ALL rows (identical
                # on every core -> bypasses the allreduce)
                nc.vector.tensor_scalar(out=csdb[:, 0, :], in0=csdb[:, 0, :],
                                        scalar1=EPS_G, scalar2=None, op0=ALU.add)
                lb2 = ep.tile([128, 2], F32)
                lnscr2 = ep.tile([128, G], F32)
                nc.scalar.activation(lnscr2[:], csdb[:, 1, :], AF.Ln,
                                     accum_out=lb2[:, 0:1])
                nc.scalar.activation(lnscr2[:], csdb[:, 0, :], AF.Ln,
                                     accum_out=lb2[:, 1:2])
                lb2r = ep.tile([128, 2], F32R)
                nc.vector.tensor_copy(lb2r[:], lb2[:])
                v23_ps = epp.tile([1, 2], F32)
                nc.tensor.matmul(v23_ps[:], ones_k[:], lb2r[:], start=True, stop=True)

                L4 = ep.tile([G, 4], F32)
                nc.sync.dma_start(out=L4[:, 0:1],
                                  in_=cc2_out[o1:o2].rearrange("(g o) -> g o", g=G))
                gs = ep.tile([G, 1], F32)
                eyescr = ep.tile([G, G], F32)
                nc.vector.scalar_tensor_tensor(out=eyescr[:], in0=bs_f[:], scalar=1.0,
                                               in1=eye64[:], op0=ALU.mult, op1=ALU.mult,
                                               accum_out=gs[:])
                neg1r = ep.tile([G, 1], F32)
                nc.vector.reduce_sum(neg1r[:], bs_f[:], axis=X)
                nc.vector.scalar_tensor_tensor(out=L4[:, 2:3], in0=neg1r[:],
                                               scalar=EPS_L, in1=gs[:], op0=ALU.add,
                                               op1=ALU.subtract)
                bs_fr = ep.tile([G, G], F32R)
                nc.vector.tensor_copy(bs_fr[:], bs_f[:])
                neg0_ps = epp.tile([G, 2], F32)
                nc.tensor.matmul(neg0_ps[:], bs_fr[:], ones2[0:G, :], start=True,
                                 stop=True)
                nc.vector.scalar_tensor_tensor(out=L4[:, 1:2], in0=neg0_ps[:, 0:1],
                                               scalar=EPS_L, in1=gs[:], op0=ALU.add,
                                               op1=ALU.subtract)
                nc.vector.scalar_tensor_tensor(out=L4[:, 3:4], in0=gs[:], scalar=EPS_L,
                                               in1=psg_f[:], op0=ALU.add,
                                               op1=ALU.subtract)
                L4ln = ep.tile([G, 4], F32)
                nc.scalar.activation(L4ln[:], L4[:], AF.Ln)
                L4r = ep.tile([G, 4], F32R)
                nc.vector.tensor_copy(L4r[:], L4ln[:])
                s4_ps = epp.tile([1, 4], F32)
                nc.tensor.matmul(s4_ps[:], ones_k[0:G, :], L4r[:], start=True, stop=True)

                vrow = ep.tile([1, 8], F32)
                nc.sync.dma_start(out=vrow[:, 0:2],
                                  in_=cc2_out[o2:o3].rearrange("(o f) -> o f", o=1))
                nc.vector.tensor_copy(vrow[:, 2:4], v23_ps[:])
                nc.vector.tensor_copy(vrow[:, 4:8], s4_ps[:])
                vscr = ep.tile([1, 8], F32)
                loss_sb = ep.tile([1, 1], F32)
                nc.vector.scalar_tensor_tensor(out=vscr[:], in0=vrow[:], scalar=1.0,
                                               in1=coef_sb[:], op0=ALU.mult, op1=ALU.mult,
                                               accum_out=loss_sb[:])
                nc.sync.dma_start(out=loss_d[:], in_=loss_sb[:])

    nc.compile()
    return nc


def kernel(**inputs):
    global LAST_RESULTS
    from concourse.bass_utils import run_bass_kernel_spmd
    import ml_dtypes

    batch = np.asarray(inputs["batch"], dtype=np.int64)
    key = batch.tobytes()
    if _PROGRAM_CACHE.get("key") != key:
        _PROGRAM_CACHE["prog"] = _build_program(batch)
        _PROGRAM_CACHE["key"] = key
    nc = _PROGRAM_CACHE["prog"]

    za = np.asarray(inputs["za"], dtype=np.float32)
    zb = np.asarray(inputs["zb"], dtype=np.float32)
    pos = np.asarray(inputs["pos"], dtype=np.float32)
    bf16 = ml_dtypes.bfloat16
    pos_bf = pos.astype(bf16)
    posT_bf = np.ascontiguousarray(pos.T).astype(bf16)
    hot = np.zeros((N, 65), dtype=bf16)
    hot[np.arange(N), batch] = 1
    hot[:, 64] = 1

    def cols(v, nb):
        return np.asarray(v, dtype=np.float32).reshape(nb, 128).T

    prm = np.zeros((128, 32), dtype=np.float32)
    prm[:, 0:4] = cols(inputs["b1"], 4)
    prm[:, 4:8] = cols(inputs["g1"], 4)
    prm[:, 8:12] = cols(inputs["be1"], 4)
    prm[:, 12:16] = cols(inputs["b2"], 4)
    prm[:, 16:20] = cols(inputs["g2"], 4)
    prm[:, 20:24] = cols(inputs["be2"], 4)
    prm[:, 24:26] = cols(inputs["b3"], 2)
    prm[:, 26] = np.float32(np.asarray(inputs["a1"]).reshape(-1)[0])
    prm[:, 27] = np.float32(np.asarray(inputs["a2"]).reshape(-1)[0])
    prm[:, 28] = np.float32(np.log(1.0 / TAU))

    coef = np.array([[-LAM / N, LAM / N, -(1.0 - LAM) / N, (1.0 - LAM) / N,
                      ALPHA / G - BETA / G, -ALPHA / (2 * G), -ALPHA / (2 * G),
                      BETA / G]], dtype=np.float32)

    shared = {
        "W1": np.asarray(inputs["W1"], dtype=np.float32),
        "W2": np.asarray(inputs["W2"], dtype=np.float32),
        "W3": np.asarray(inputs["W3"], dtype=np.float32),
        "prm": prm, "coef": coef,
    }
    in_maps = []
    for c in range(NC):
        sl = slice(c * R, (c + 1) * R)
        m = dict(shared)
        m["za_s"] = za[sl]
        m["zb_s"] = zb[sl]
        m["pos_s"] = pos_bf[sl]
        m["posT_s"] = posT_bf[sl]
        m["hot_s"] = hot[sl]
        in_maps.append(m)

    res = run_bass_kernel_spmd(nc, in_maps, list(range(NC)))
    LAST_RESULTS = res
    out = np.concatenate([res.results[c]["out_s"] for c in range(NC)], axis=0)
    loss = np.float32(res.results[0]["loss"][0, 0])
    return loss, out


# revision 25
# speedup vs baseline: 1.0588x; 1.0588x over previous
"""Bass/Trainium2 kernel for nn_EnhancedContrast (8-core SPMD).

Sharding: rows (N=8192) split across 8 cores, 1024 rows each. Each core:
  - runs the projection MLP on its za/zb row-slab (activation-transposed layout,
    zb first so the single all-gather of normalized zb overlaps za's MLP),
  - computes its row-slab of m = exp(cos/tau): rowsum/dot_a reduce locally along
    the free dim; colsum partials ride a ones-column on the one-hot matmul and
    dot_b partials come from a host-transposed bf16 pos^T via a ones-matmul
    partition reduction,
  - one AllReduce (~84KB) combines batch_sim/pos_sim_graph/colsum/dot_b/log
    partials; every core then computes the identical scalar loss.
"""
import sys

sys.path.insert(0, "/opt/trn_rl_repo")

import numpy as np

N, H, D, G = 8192, 512, 256, 64
TAU, LAM, ALPHA, BETA = 0.5, 0.5, 1.0, 1.0
EPS_G, EPS_L = 1e-6, 1e-5
NC = 8            # cores
R = N // NC       # rows per core = 1024
NB = R // 128     # 128-row blocks per core = 8
HC = H // 128     # hidden chunks = 4
DC = D // 128     # proj-dim chunks = 2
NS = R // 512     # 512-wide moving slices per 1024 = 2

LAST_RESULTS = None  # stashed BassKernelResults for test.py
_PROGRAM_CACHE = {}


def _build_program(batch_np):
    import concourse.mybir as mybir
    import concourse.tile as tile
    from concourse import bacc
    from concourse.masks import make_identity

    F32 = mybir.dt.float32
    F32R = mybir.dt.float32r
    BF16 = mybir.dt.bfloat16
    I32 = mybir.dt.int32
    AF = mybir.ActivationFunctionType
    ALU = mybir.AluOpType
    X = mybir.AxisListType.X

    # group segments along the full column axis (batch is sorted)
    bounds = np.searchsorted(batch_np, np.arange(G + 1))
    segs_by_jc = [[] for _ in range(NB)]
    for g in range(G):
        lo, hi = int(bounds[g]), int(bounds[g + 1])
        first = True
        j = lo
        while j < hi:
            jc = j // R
            e = min(hi, (jc + 1) * R)
            segs_by_jc[jc].append((g, j - jc * R, e - jc * R, first))
            first = False
            j = e

    nc = bacc.Bacc("TRN2", target_bir_lowering=False, debug=False, num_devices=NC)

    # ---- I/O ----
    za_d = nc.dram_tensor("za_s", [R, H], F32, kind="ExternalInput")
    zb_d = nc.dram_tensor("zb_s", [R, H], F32, kind="ExternalInput")
    pos_d = nc.dram_tensor("pos_s", [R, N], BF16, kind="ExternalInput")
    posT_d = nc.dram_tensor("posT_s", [R, N], BF16, kind="ExternalInput")
    hot_d = nc.dram_tensor("hot_s", [R, 65], BF16, kind="ExternalInput")
    w1_d = nc.dram_tensor("W1", [H, H], F32R, kind="ExternalInput")
    w2_d = nc.dram_tensor("W2", [H, H], F32R, kind="ExternalInput")
    w3_d = nc.dram_tensor("W3", [H, D], F32R, kind="ExternalInput")
    prm_d = nc.dram_tensor("prm", [128, 32], F32, kind="ExternalInput")
    coef_d = nc.dram_tensor("coef", [1, 8], F32, kind="ExternalInput")
    out_d = nc.dram_tensor("out_s", [R, 2 * D], F32, kind="ExternalOutput")
    loss_d = nc.dram_tensor("loss", [1, 1], F32, kind="ExternalOutput")

    with tile.TileContext(nc) as tc:
        with tc.tile_pool(name="consts", bufs=1) as consts, \
             tc.tile_pool(name="live", bufs=1) as live, \
             tc.tile_pool(name="dram", bufs=1, space="DRAM") as dram:
            # packed per-partition params: b1c g1c be1c b2c g2c be2c (4 cols each),
            # b3c (2), a1, a2, ln2 -> 29 cols used
            prm = consts.tile([128, 32], F32)
            nc.sync.dma_start(out=prm[:], in_=prm_d[:])
            b1c, g1c, be1c = prm[:, 0:4], prm[:, 4:8], prm[:, 8:12]
            b2c, g2c, be2c = prm[:, 12:16], prm[:, 16:20], prm[:, 20:24]
            b3c = prm[:, 24:26]
            a1_bc, a2_bc = prm[:, 26:27], prm[:, 27:28]
            ln2_t = prm[0:1, 28:29]
            coef_sb = consts.tile([1, 8], F32)
            nc.sync.dma_start(out=coef_sb[:], in_=coef_d[:])

            ident = consts.tile([128, 128], F32)
            make_identity(nc, ident[:])
            ones_k32 = consts.tile([128, 1], F32)
            nc.vector.memset(ones_k32[:], 1.0)
            ones_k = consts.tile([128, 1], F32R)
            nc.vector.tensor_copy(ones_k[:], ones_k32[:])
            ones_kb = consts.tile([128, 1], BF16)
            nc.vector.tensor_copy(ones_kb[:], ones_k32[:])
            ones2_32 = consts.tile([128, 2], F32)
            nc.vector.memset(ones2_32[:], 1.0)
            ones2 = consts.tile([128, 2], F32R)
            nc.vector.tensor_copy(ones2[:], ones2_32[:])

            ab1 = consts.tile([128, HC], F32)
            nc.vector.tensor_scalar(out=ab1[:], in0=b1c, scalar1=a1_bc, scalar2=None,
                                    op0=ALU.mult)
            ab2 = consts.tile([128, HC], F32)
            nc.vector.tensor_scalar(out=ab2[:], in0=b2c, scalar1=a2_bc, scalar2=None,
                                    op0=ALU.mult)

            # one-hot (64 groups + ones column) per row-block, host-prepared
            hot_r = consts.tile([128, NB, 65], BF16)
            nc.sync.dma_start(out=hot_r[:],
                              in_=hot_d[:].rearrange("(b p) g -> p b g", p=128))

            # eye64 for diag extraction in the epilogue
            iota_i = consts.tile([1, G], I32)
            nc.gpsimd.iota(iota_i[:], pattern=[[1, G]], base=0, channel_multiplier=0)
            iota_bc_i = consts.tile([128, G], I32)
            nc.gpsimd.partition_broadcast(iota_bc_i[:], iota_i[:])
            iota_f = consts.tile([128, G], F32)
            nc.vector.tensor_copy(iota_f[:], iota_bc_i[:])
            iota_col_i = consts.tile([128, 1], I32)
            nc.gpsimd.iota(iota_col_i[:], pattern=[[0, 1]], base=0, channel_multiplier=1)
            iota_col = consts.tile([128, 1], F32)
            nc.vector.tensor_copy(iota_col[:], iota_col_i[:])
            eye64 = consts.tile([G, G], F32)
            nc.vector.tensor_scalar(out=eye64[:], in0=iota_f[0:G, :],
                                    scalar1=iota_col[0:G, :], scalar2=None,
                                    op0=ALU.is_equal)

            # long-lived similarity-phase tensors
            za_r = live.tile([128, DC, R], BF16)     # stationary for the m-slab matmuls
            invna_cols = live.tile([128, NB], F32)
            invnb_cols = live.tile([128, NB], F32)
            rsacc = live.tile([128, NB * NB], F32)   # rowsum partials, col = ib*NB+jc
            daacc = live.tile([128, NB * NB], F32)
            bs_sb = live.tile([G, G], F32)
            nc.vector.memset(bs_sb[:], 0.0)
            psg_sb = live.tile([G, 1], F32)
            v01_sb = live.tile([1, 2], F32)
            ps_cols_r = live.tile([128, NB, 2], BF16)

            # collective buffers
            ag_in = dram.tile([DC, 128, R], BF16)
            ag_out = dram.tile([NC, DC, 128, R], BF16, addr_space="Shared")
            o1, o2, o3, o4 = G * G, G * G + G, G * G + G + 2, G * G + G + 2 + N
            CC2 = o4 + N
            cc2_in = dram.tile([CC2], F32)
            cc2_out = dram.tile([CC2], F32, addr_space="Shared")

            # ------------- Phase A: MLP (zb first; its gather overlaps za) -------------
            with tc.tile_pool(name="wpool", bufs=1) as wp, \
                 tc.tile_pool(name="mlp", bufs=1) as mp, \
                 tc.tile_pool(name="mps", bufs=2, space="PSUM") as mpp, \
                 tc.tile_pool(name="tps", bufs=2, space="PSUM") as tpp, \
                 tc.tile_pool(name="stg", bufs=1) as sg, \
                 tc.tile_pool(name="npsn", bufs=2, space="PSUM") as npn:
                w1 = wp.tile([128, HC, H], F32R)
                nc.sync.dma_start(out=w1[:], in_=w1_d[:].rearrange("(c p) o -> p c o", p=128))
                w2 = wp.tile([128, HC, H], F32R)
                nc.sync.dma_start(out=w2[:], in_=w2_d[:].rearrange("(c p) o -> p c o", p=128))
                w3 = wp.tile([128, HC, D], F32R)
                nc.sync.dma_start(out=w3[:], in_=w3_d[:].rearrange("(c p) o -> p c o", p=128))

                def load_xT(side, x_d):
                    xT = mp.tile([128, HC, R], F32R, tag="big", bufs=4, name=f"xT{side}")
                    for rb in range(NB):
                        st = sg.tile([128, H], F32, tag="stage", bufs=3)
                        nc.sync.dma_start(out=st[:], in_=x_d[rb * 128:(rb + 1) * 128, :])
                        tp4 = tpp.tile([128, H], F32, tag="tp")
                        for c in range(HC):
                            nc.tensor.transpose(tp4[:, c * 128:(c + 1) * 128],
                                                st[:, c * 128:(c + 1) * 128], ident[:])
                        nc.vector.tensor_copy(
                            xT[:, :, rb * 128:(rb + 1) * 128],
                            tp4[:].rearrange("p (c q) -> p c q", c=HC))
                    return xT

                def layer(x_in, w, n_ob, act_fn, scale, bias_cols, gc, bec, out_tag,
                          out_dtype):
                    bufs = 4 if out_tag == "big" else 1
                    out_t = mp.tile([128, n_ob, R], out_dtype, tag=out_tag, bufs=bufs,
                                    name=f"L{out_tag}{n_ob}")
                    for ob in range(n_ob):
                        ps = mpp.tile([128, R], F32, tag="mm")
                        for ns in range(NS):
                            sl = slice(ns * 512, (ns + 1) * 512)
                            for c in range(HC):
                                nc.tensor.matmul(ps[:, sl],
                                                 w[:, c, ob * 128:(ob + 1) * 128],
                                                 x_in[:, c, sl], start=(c == 0),
                                                 stop=(c == HC - 1))
                        if gc is not None:
                            th = sg.tile([128, R], F32, tag="th", bufs=2)
                            nc.scalar.activation(th[:], ps[:], act_fn, scale=scale,
                                                 bias=bias_cols[:, ob:ob + 1])
                            nc.vector.tensor_scalar(out=out_t[:, ob, :], in0=th[:],
                                                    scalar1=gc[:, ob:ob + 1],
                                                    scalar2=bec[:, ob:ob + 1],
                                                    op0=ALU.mult, op1=ALU.add)
                        else:
                            nc.scalar.activation(out_t[:, ob, :], ps[:], act_fn,
                                                 scale=scale,
                                                 bias=bias_cols[:, ob:ob + 1])
                    return out_t

                zpT = {}
                for side, x_d in (("b", zb_d), ("a", za_d)):
                    xT = load_xT(side, x_d)
                    h1 = layer(xT, w1, HC, AF.Tanh, a1_bc, ab1, g1c, be1c, "big", F32R)
                    h2 = layer(h1, w2, HC, AF.Tanh, a2_bc, ab2, g2c, be2c, "big", F32R)
                    zpT[side] = layer(h2, w3, DC, AF.Silu, 1.0, b3c, None, None,
                                      f"zp{side}", F32)

                    sq = sg.tile([128, DC, R], F32R, tag="sq")
                    nc.vector.tensor_mul(sq[:], zpT[side][:], zpT[side][:])
                    lnr = sg.tile([1, R], F32, tag="lnr")
                    for ns in range(NS):
                        sl = slice(ns * 512, (ns + 1) * 512)
                        ns2 = npn.tile([1, 512], F32, tag="smallps", name="ns2")
                        for c in range(DC):
                            nc.tensor.matmul(ns2[:], ones_k[:], sq[:, c, sl],
                                             start=(c == 0), stop=(c == DC - 1))
                        nc.scalar.activation(lnr[:, sl], ns2[:], AF.Ln)
                    if side == "b":
                        # zb_hat = zb_pT * (invnb/tau)[r]; gather it (the only gather)
                        invnt = sg.tile([1, R], F32, tag="invnt")
                        nc.scalar.activation(invnt[:], lnr[:], AF.Exp, scale=-0.5,
                                             bias=ln2_t)
                        bc = sg.tile([128, R], F32, tag="bc")
                        nc.gpsimd.partition_broadcast(bc[:], invnt[:])
                        hat = sg.tile([128, DC, R], BF16, tag="hat")
                        for c in range(DC):
                            nc.vector.tensor_mul(hat[:, c, :], zpT[side][:, c, :], bc[:])
                        nc.sync.dma_start(out=ag_in[:].rearrange("c p r -> p c r"),
                                          in_=hat[:])
                        nc.gpsimd.collective_compute(
                            "AllGather", ALU.bypass, replica_groups=[list(range(NC))],
                            ins=[ag_in[:].opt()], outs=[ag_out[:].opt()])
                        # invnb cols (for the pos_sim diag path): invnb = invnt/2
                        dnormb = dram.tile([1, R], F32, tag="dnormb")
                        nc.sync.dma_start(out=dnormb[:], in_=invnt[:])
                        ivb = sg.tile([128, NB], F32, tag="ivb")
                        nc.sync.dma_start(
                            out=ivb[:],
                            in_=dnormb[0:1, :].rearrange("o (b p) -> (o p) b", b=NB))
                        nc.vector.tensor_scalar(out=invnb_cols[:], in0=ivb[:],
                                                scalar1=float(TAU), scalar2=None,
                                                op0=ALU.mult)
                    else:
                        # za stays raw; invna applied at exp time (per-partition scale)
                        invn = sg.tile([1, R], F32, tag="invnt")
                        nc.scalar.activation(invn[:], lnr[:], AF.Exp, scale=-0.5)
                        dnorma = dram.tile([1, R], F32, tag="dnorma")
                        nc.sync.dma_start(out=dnorma[:], in_=invn[:])
                        nc.sync.dma_start(
                            out=invna_cols[:],
                            in_=dnorma[0:1, :].rearrange("o (b p) -> (o p) b", b=NB))

                nc.vector.tensor_copy(za_r[:], zpT["a"][:])

                # output slab: transpose zpT back to natural and store
                for rb in range(NB):
                    tp4 = tpp.tile([128, 2 * D], F32, tag="tp")
                    for k, side in enumerate(("a", "b")):
                        for ob in range(DC):
                            nc.tensor.transpose(
                                tp4[:, k * D + ob * 128:k * D + (ob + 1) * 128],
                                zpT[side][:, ob, rb * 128:(rb + 1) * 128], ident[:])
                    ost = sg.tile([128, 2 * D], F32, tag="ost", bufs=2)
                    nc.vector.tensor_copy(ost[:], tp4[:])
                    nc.sync.dma_start(out=out_d[rb * 128:(rb + 1) * 128, :], in_=ost[:])

                # pos_sim (diag of m): exp(rawdot*invna*invnb/tau); psg partial
                prod = sg.tile([128, DC, R], F32R, tag="sq")
                nc.vector.tensor_mul(prod[:], zpT["a"][:], zpT["b"][:])
                rd_ps = npn.tile([128, NB, 2], F32, tag="smallps", name="rd_ps")
                for ib in range(NB):
                    for c in range(DC):
                        nc.tensor.matmul(rd_ps[:, ib, :],
                                         prod[:, c, ib * 128:(ib + 1) * 128],
                                         ones2[:], start=(c == 0), stop=(c == DC - 1))
                t1 = sg.tile([128, NB], F32, tag="t1")
                nc.vector.tensor_mul(t1[:], rd_ps[:, :, 0], invna_cols[:])
                t2 = sg.tile([128, NB], F32, tag="t2")
                nc.vector.tensor_mul(t2[:], t1[:], invnb_cols[:])
                ps_cols = sg.tile([128, NB], F32, tag="t3")
                nc.scalar.activation(ps_cols[:], t2[:], AF.Exp, scale=float(1.0 / TAU))
                zcols = sg.tile([128, NB], F32, tag="zcols")
                nc.vector.memset(zcols[:], 0.0)
                nc.vector.tensor_copy(ps_cols_r[:, :, 1], zcols[:])
                nc.vector.tensor_copy(ps_cols_r[:, :, 0], ps_cols[:])
                psg_ps = npn.tile([G, 2], F32, tag="smallps", name="psg_ps")
                for ib in range(NB):
                    nc.tensor.matmul(psg_ps[:], hot_r[:, ib, 0:G], ps_cols_r[:, ib, :],
                                     start=(ib == 0), stop=(ib == NB - 1))
                nc.vector.tensor_copy(psg_sb[:], psg_ps[:, 0:1])

            # ---------------- Phase C: similarity slab ----------------
            with tc.tile_pool(name="strm", bufs=3) as strm, \
                 tc.tile_pool(name="pospool", bufs=3) as pp, \
                 tc.tile_pool(name="mpool", bufs=3) as mpo, \
                 tc.tile_pool(name="scr", bufs=2) as scp, \
                 tc.tile_pool(name="sps", bufs=2, space="PSUM") as sps, \
                 tc.tile_pool(name="ups", bufs=2, space="PSUM") as ups, \
                 tc.tile_pool(name="tiny", bufs=4) as tiny:
                for jc in range(NB):
                    zb_sl = strm.tile([128, DC, R], BF16, tag="zb_sl", name="zb_sl")
                    nc.sync.dma_start(out=zb_sl[:],
                                      in_=ag_out[jc].rearrange("c p r -> p c r"))
                    # u rows 0..64: [hot|ones]^T @ m  (row 64 = colsum partial);
                    # row 96: ones^T @ (m * posT) = dot_b partial
                    u_ps = ups.tile([128, R], F32, tag="u", name="u_ps")
                    for ib in range(NB):
                        ibs = slice(ib * 128, (ib + 1) * 128)
                        acol = ib * NB + jc
                        pos_t = pp.tile([128, R], BF16, tag="pos", name="pos_t")
                        nc.gpsimd.dma_start(out=pos_t[:],
                                            in_=pos_d[ibs, jc * R:(jc + 1) * R])
                        posT_t = pp.tile([128, R], BF16, tag="posT", name="posT_t")
                        nc.gpsimd.dma_start(out=posT_t[:],
                                            in_=posT_d[ibs, jc * R:(jc + 1) * R])
                        sa = sps.tile([128, R], F32, tag="S", name="sa")
                        for ns in range(NS):
                            sl = slice(ns * 512, (ns + 1) * 512)
                            for c in range(DC):
                                nc.tensor.matmul(sa[:, sl], za_r[:, c, ibs],
                                                 zb_sl[:, c, sl],
                                                 start=(c == 0), stop=(c == DC - 1))
                        m_a = mpo.tile([128, R], BF16, tag="ma", name="m_a")
                        nc.scalar.activation(m_a[:], sa[:], AF.Exp,
                                             scale=invna_cols[:, ib:ib + 1],
                                             accum_out=rsacc[:, acol:acol + 1])
                        scr_a = scp.tile([128, R], BF16, tag="scra", name="scr_a")
                        nc.vector.scalar_tensor_tensor(
                            out=scr_a[:], in0=m_a[:], scalar=1.0, in1=pos_t[:],
                            op0=ALU.mult, op1=ALU.mult,
                            accum_out=daacc[:, acol:acol + 1])
                        prod_b = scp.tile([128, R], BF16, tag="prodb", name="prod_b")
                        nc.vector.tensor_mul(prod_b[:], m_a[:], posT_t[:])
                        for ns in range(NS):
                            sl = slice(ns * 512, (ns + 1) * 512)
                            nc.tensor.matmul(u_ps[0:65, sl], hot_r[:, ib, :], m_a[:, sl],
                                             start=(ib == 0), stop=(ib == NB - 1),
                                             skip_group_check=True)
                            nc.tensor.matmul(u_ps[96:97, sl], ones_kb[:], prod_b[:, sl],
                                             start=(ib == 0), stop=(ib == NB - 1),
                                             skip_group_check=True,
                                             tile_position=(0, 96))
                    csdb_st = scp.tile([128, R], F32, tag="csdb", name="csdb_st",
                                       bufs=2)
                    nc.scalar.copy(csdb_st[64:65, :], u_ps[64:65, :])
                    nc.scalar.copy(csdb_st[96:97, :], u_ps[96:97, :])
                    nc.sync.dma_start(
                        out=cc2_in[o3 + jc * R:o3 + (jc + 1) * R].rearrange(
                            "(o f) -> o f", o=1),
                        in_=csdb_st[64:65, :])
                    nc.sync.dma_start(
                        out=cc2_in[o4 + jc * R:o4 + (jc + 1) * R].rearrange(
                            "(o f) -> o f", o=1),
                        in_=csdb_st[96:97, :])
                    for (g, lo, hi, first) in segs_by_jc[jc]:
                        if first:
                            nc.vector.reduce_sum(bs_sb[:, g:g + 1], u_ps[0:G, lo:hi],
                                                 axis=X)
                        else:
                            tmp = tiny.tile([G, 1], F32, tag="segtmp", name="segtmp")
                            nc.vector.reduce_sum(tmp[:], u_ps[0:G, lo:hi], axis=X)
                            nc.vector.tensor_add(bs_sb[:, g:g + 1], bs_sb[:, g:g + 1],
                                                 tmp[:])

            # ---------------- Phase D: local log-sums + allreduce ----------------
            with tc.tile_pool(name="ep", bufs=1) as ep, \
                 tc.tile_pool(name="eps", bufs=2, space="PSUM") as epp:
                red = ep.tile([128, 2, NB], F32)
                for k, acc in enumerate((daacc, rsacc)):
                    nc.vector.reduce_sum(red[:, k, :],
                                         acc[:].rearrange("p (ib jc) -> p ib jc", ib=NB),
                                         axis=X)
                nc.vector.tensor_scalar(out=red[:, 1, :], in0=red[:, 1, :],
                                        scalar1=EPS_G, scalar2=None, op0=ALU.add)
                la2 = ep.tile([128, 2], F32)
                lnscr = ep.tile([128, NB], F32)
                for k in range(2):
                    nc.scalar.activation(lnscr[:], red[:, k, :], AF.Ln,
                                         accum_out=la2[:, k:k + 1])
                la2r = ep.tile([128, 2], F32R)
                nc.vector.tensor_copy(la2r[:], la2[:])
                v01_ps = epp.tile([1, 2], F32)
                nc.tensor.matmul(v01_ps[:], ones_k[:], la2r[:], start=True, stop=True)
                nc.vector.tensor_copy(v01_sb[:], v01_ps[:])

                nc.sync.dma_start(out=cc2_in[0:o1].rearrange("(g h) -> g h", g=G),
                                  in_=bs_sb[:])
                nc.sync.dma_start(out=cc2_in[o1:o2].rearrange("(g o) -> g o", g=G),
                                  in_=psg_sb[:])
                nc.sync.dma_start(out=cc2_in[o2:o3].rearrange("(o f) -> o f", o=1),
                                  in_=v01_sb[:])
                nc.gpsimd.collective_compute(
                    "AllReduce", ALU.add, replica_groups=[list(range(NC))],
                    ins=[cc2_in[:].opt()], outs=[cc2_out[:].opt()])

                # ---------------- Phase E: final scalar loss ----------------
                bs_f = ep.tile([G, G], F32)
                nc.sync.dma_start(out=bs_f[:],
                                  in_=cc2_out[0:o1].rearrange("(g h) -> g h", g=G))
                psg_f = ep.tile([G, 1], F32)
                nc.sync.dma_start(out=psg_f[:],
                                  in_=cc2_out[o1:o2].rearrange("(g o) -> g o", g=G))
                # full colsum/dot_b rows -> [128, 64] col layout
                csdb = ep.tile([128, 2, G], F32)
                nc.sync.dma_start(out=csdb[:, 0, :],
                                  in_=cc2_out[o3:o4].rearrange("(b p) -> p b", p=128))
                nc.sync.dma_start(out=csdb[:, 1, :],
                                  in_=cc2_out[o4:].rearrange("(b p) -> p b", p=128))
                # v2 = sum ln(dot_b), v3 = sum ln(colsum+eps) over ALL rows (identical
                # on every core -> bypasses the allreduce)
                nc.vector.tensor_scalar(out=csdb[:, 0, :], in0=csdb[:, 0, :],
                                        scalar1=EPS_G, scalar2=None, op0=ALU.add)
                lb2 = ep.tile([128, 2], F32)
                lnscr2 = ep.tile([128, G], F32)
                nc.scalar.activation(lnscr2[:], csdb[:, 1, :], AF.Ln,
                                     accum_out=lb2[:, 0:1])
                nc.scalar.activation(lnscr2[:], csdb[:, 0, :], AF.Ln,
                                     accum_out=lb2[:, 1:2])
                lb2r = ep.tile([128, 2], F32R)
                nc.vector.tensor_copy(lb2r[:], lb2[:])
                v23_ps = epp.tile([1, 2], F32)
                nc.tensor.matmul(v23_ps[:], ones_k[:], lb2r[:], start=True, stop=True)

                L4 = ep.tile([G, 4], F32)
                nc.sync.dma_start(out=L4[:, 0:1],
                                  in_=cc2_out[o1:o2].rearrange("(g o) -> g o", g=G))
                gs = ep.tile([G, 1], F32)
                eyescr = ep.tile([G, G], F32)
                nc.vector.scalar_tensor_tensor(out=eyescr[:], in0=bs_f[:], scalar=1.0,
                                               in1=eye64[:], op0=ALU.mult, op1=ALU.mult,
                                               accum_out=gs[:])
                neg1r = ep.tile([G, 1], F32)
                nc.vector.reduce_sum(neg1r[:], bs_f[:], axis=X)
                nc.vector.scalar_tensor_tensor(out=L4[:, 2:3], in0=neg1r[:],
                                               scalar=EPS_L, in1=gs[:], op0=ALU.add,
                                               op1=ALU.subtract)
                bs_fr = ep.tile([G, G], F32R)
                nc.vector.tensor_copy(bs_fr[:], bs_f[:])
                neg0_ps = epp.tile([G, 2], F32)
                nc.tensor.matmul(neg0_ps[:], bs_fr[:], ones2[0:G, :], start=True,
                                 stop=True)
                nc.vector.scalar_tensor_tensor(out=L4[:, 1:2], in0=neg0_ps[:, 0:1],
                                               scalar=EPS_L, in1=gs[:], op0=ALU.add,
                                               op1=ALU.subtract)
                nc.vector.scalar_tensor_tensor(out=L4[:, 3:4], in0=gs[:], scalar=EPS_L,
                                               in1=psg_f[:], op0=ALU.add,
                                               op1=ALU.subtract)
                L4ln = ep.tile([G, 4], F32)
                nc.scalar.activation(L4ln[:], L4[:], AF.Ln)
                L4r = ep.tile([G, 4], F32R)
                nc.vector.tensor_copy(L4r[:], L4ln[:])
                s4_ps = epp.tile([1, 4], F32)
                nc.tensor.matmul(s4_ps[:], ones_k[0:G, :], L4r[:], start=True, stop=True)

                vrow = ep.tile([1, 8], F32)
                nc.sync.dma_start(out=vrow[:, 0:2],
                                  in_=cc2_out[o2:o3].rearrange("(o f) -> o f", o=1))
                nc.vector.tensor_copy(vrow[:, 2:4], v23_ps[:])
                nc.vector.tensor_copy(vrow[:, 4:8], s4_ps[:])
                vscr = ep.tile([1, 8], F32)
                loss_sb = ep.tile([1, 1], F32)
                nc.vector.scalar_tensor_tensor(out=vscr[:], in0=vrow[:], scalar=1.0,
                                               in1=coef_sb[:], op0=ALU.mult, op1=ALU.mult,
                                               accum_out=loss_sb[:])
                nc.sync.dma_start(out=loss_d[:], in_=loss_sb[:])

    nc.compile()
    return nc


def kernel(**inputs):
    global LAST_RESULTS
    from concourse.bass_utils import run_bass_kernel_spmd
    import ml_dtypes

    batch = np.asarray(inputs["batch"], dtype=np.int64)
    key = batch.tobytes()
    if _PROGRAM_CACHE.get("key") != key:
        _PROGRAM_CACHE["prog"] = _build_program(batch)
        _PROGRAM_CACHE["key"] = key
    nc = _PROGRAM_CACHE["prog"]

    za = np.asarray(inputs["za"], dtype=np.float32)
    zb = np.asarray(inputs["zb"], dtype=np.float32)
    pos = np.asarray(inputs["pos"], dtype=np.float32)
    bf16 = ml_dtypes.bfloat16
    pos_bf = pos.astype(bf16)
    posT_bf = np.ascontiguousarray(pos.T).astype(bf16)
    hot = np.zeros((N, 65), dtype=bf16)
    hot[np.arange(N), batch] = 1
    hot[:, 64] = 1

    def cols(v, nb):
        return np.asarray(v, dtype=np.float32).reshape(nb, 128).T

    prm = np.zeros((128, 32), dtype=np.float32)
    prm[:, 0:4] = cols(inputs["b1"], 4)
    prm[:, 4:8] = cols(inputs["g1"], 4)
    prm[:, 8:12] = cols(inputs["be1"], 4)
    prm[:, 12:16] = cols(inputs["b2"], 4)
    prm[:, 16:20] = cols(inputs["g2"], 4)
    prm[:, 20:24] = cols(inputs["be2"], 4)
    prm[:, 24:26] = cols(inputs["b3"], 2)
    prm[:, 26] = np.float32(np.asarray(inputs["a1"]).reshape(-1)[0])
    prm[:, 27] = np.float32(np.asarray(inputs["a2"]).reshape(-1)[0])
    prm[:, 28] = np.float32(np.log(1.0 / TAU))

    coef = np.array([[-LAM / N, LAM / N, -(1.0 - LAM) / N, (1.0 - LAM) / N,
                      ALPHA / G - BETA / G, -ALPHA / (2 * G), -ALPHA / (2 * G),
                      BETA / G]], dtype=np.float32)

    shared = {
        "W1": np.asarray(inputs["W1"], dtype=np.float32),
        "W2": np.asarray(inputs["W2"], dtype=np.float32),
        "W3": np.asarray(inputs["W3"], dtype=np.float32),
        "prm": prm, "coef": coef,
    }
    in_maps = []
    for c in range(NC):
        sl = slice(c * R, (c + 1) * R)
        m = dict(shared)
        m["za_s"] = za[sl]
        m["zb_s"] = zb[sl]
        m["pos_s"] = pos_bf[sl]
        m["posT_s"] = posT_bf[sl]
        m["hot_s"] = hot[sl]
        in_maps.append(m)

    res = run_bass_kernel_spmd(nc, in_maps, list(range(NC)))
    LAST_RESULTS = res
    out = np.concatenate([res.results[c]["out_s"] for c in range(NC)], axis=0)
    loss = np.float32(res.results[0]["loss"][0, 0])
    return loss, out


# revision 26
# speedup vs baseline: 1.0819x; 1.0217x over previous
"""Bass/Trainium2 kernel for nn_EnhancedContrast (8-core SPMD).

Sharding: rows (N=8192) split across 8 cores, 1024 rows each. Each core:
  - runs the projection MLP on its za/zb row-slab (activation-transposed layout,
    zb first so the single all-gather of normalized zb overlaps za's MLP),
  - computes its row-slab of m = exp(cos/tau): rowsum/dot_a reduce locally along
    the free dim; colsum partials ride a ones-column on the one-hot matmul and
    dot_b partials come from a host-transposed bf16 pos^T via a ones-matmul
    partition reduction,
  - one AllReduce (~84KB) combines batch_sim/pos_sim_graph/colsum/dot_b/log
    partials; every core then computes the identical scalar loss.
"""
import sys

sys.path.insert(0, "/opt/trn_rl_repo")

import numpy as np

N, H, D, G = 8192, 512, 256, 64
TAU, LAM, ALPHA, BETA = 0.5, 0.5, 1.0, 1.0
EPS_G, EPS_L = 1e-6, 1e-5
NC = 8            # cores
R = N // NC       # rows per core = 1024
NB = R // 128     # 128-row blocks per core = 8
HC = H // 128     # hidden chunks = 4
DC = D // 128     # proj-dim chunks = 2
NS = R // 512     # 512-wide moving slices per 1024 = 2

LAST_RESULTS = None  # stashed BassKernelResults for test.py
_PROGRAM_CACHE = {}


def _build_program(batch_np):
    import concourse.mybir as mybir
    import concourse.tile as tile
    from concourse import bacc
    from concourse.masks import make_identity

    F32 = mybir.dt.float32
    F32R = mybir.dt.float32r
    BF16 = mybir.dt.bfloat16
    I32 = mybir.dt.int32
    AF = mybir.ActivationFunctionType
    ALU = mybir.AluOpType
    X = mybir.AxisListType.X

    # group segments along the full column axis (batch is sorted)
    bounds = np.searchsorted(batch_np, np.arange(G + 1))
    segs_by_jc = [[] for _ in range(NB)]
    for g in range(G):
        lo, hi = int(bounds[g]), int(bounds[g + 1])
        first = True
        j = lo
        while j < hi:
            jc = j // R
            e = min(hi, (jc + 1) * R)
            segs_by_jc[jc].append((g, j - jc * R, e - jc * R, first))
            first = False
            j = e

    nc = bacc.Bacc("TRN2", target_bir_lowering=False, debug=False, num_devices=NC)

    # ---- I/O ----
    za_d = nc.dram_tensor("za_s", [R, H], F32, kind="ExternalInput")
    zb_d = nc.dram_tensor("zb_s", [R, H], F32, kind="ExternalInput")
    pos_d = nc.dram_tensor("pos_s", [R, N], BF16, kind="ExternalInput")
    posT_d = nc.dram_tensor("posT_s", [R, N], BF16, kind="ExternalInput")
    hot_d = nc.dram_tensor("hot_s", [R, 65], BF16, kind="ExternalInput")
    w1_d = nc.dram_tensor("W1", [H, H], F32R, kind="ExternalInput")
    w2_d = nc.dram_tensor("W2", [H, H], F32R, kind="ExternalInput")
    w3_d = nc.dram_tensor("W3", [H, D], F32R, kind="ExternalInput")
    prm_d = nc.dram_tensor("prm", [128, 32], F32, kind="ExternalInput")
    coef_d = nc.dram_tensor("coef", [1, 8], F32, kind="ExternalInput")
    out_d = nc.dram_tensor("out_s", [R, 2 * D], F32, kind="ExternalOutput")
    loss_d = nc.dram_tensor("loss", [1, 1], F32, kind="ExternalOutput")

    with tile.TileContext(nc) as tc:
        with tc.tile_pool(name="consts", bufs=1) as consts, \
             tc.tile_pool(name="live", bufs=1) as live, \
             tc.tile_pool(name="dram", bufs=1, space="DRAM") as dram:
            # packed per-partition params: b1c g1c be1c b2c g2c be2c (4 cols each),
            # b3c (2), a1, a2, ln2 -> 29 cols used
            prm = consts.tile([128, 32], F32)
            nc.sync.dma_start(out=prm[:], in_=prm_d[:])
            b1c, g1c, be1c = prm[:, 0:4], prm[:, 4:8], prm[:, 8:12]
            b2c, g2c, be2c = prm[:, 12:16], prm[:, 16:20], prm[:, 20:24]
            b3c = prm[:, 24:26]
            a1_bc, a2_bc = prm[:, 26:27], prm[:, 27:28]
            ln2_t = prm[0:1, 28:29]
            coef_sb = consts.tile([1, 8], F32)
            nc.sync.dma_start(out=coef_sb[:], in_=coef_d[:])

            ident = consts.tile([128, 128], F32)
            make_identity(nc, ident[:])
            ones_k32 = consts.tile([128, 1], F32)
            nc.vector.memset(ones_k32[:], 1.0)
            ones_k = consts.tile([128, 1], F32R)
            nc.vector.tensor_copy(ones_k[:], ones_k32[:])
            ones_kb = consts.tile([128, 1], BF16)
            nc.vector.tensor_copy(ones_kb[:], ones_k32[:])
            ones2_32 = consts.tile([128, 2], F32)
            nc.vector.memset(ones2_32[:], 1.0)
            ones2 = consts.tile([128, 2], F32R)
            nc.vector.tensor_copy(ones2[:], ones2_32[:])

            ab1 = consts.tile([128, HC], F32)
            nc.vector.tensor_scalar(out=ab1[:], in0=b1c, scalar1=a1_bc, scalar2=None,
                                    op0=ALU.mult)
            ab2 = consts.tile([128, HC], F32)
            nc.vector.tensor_scalar(out=ab2[:], in0=b2c, scalar1=a2_bc, scalar2=None,
                                    op0=ALU.mult)

            # one-hot (64 groups + ones column) per row-block, host-prepared
            hot_r = consts.tile([128, NB, 65], BF16)
            nc.sync.dma_start(out=hot_r[:],
                              in_=hot_d[:].rearrange("(b p) g -> p b g", p=128))

            # eye64 for diag extraction in the epilogue
            iota_i = consts.tile([1, G], I32)
            nc.gpsimd.iota(iota_i[:], pattern=[[1, G]], base=0, channel_multiplier=0)
            iota_bc_i = consts.tile([128, G], I32)
            nc.gpsimd.partition_broadcast(iota_bc_i[:], iota_i[:])
            iota_f = consts.tile([128, G], F32)
            nc.vector.tensor_copy(iota_f[:], iota_bc_i[:])
            iota_col_i = consts.tile([128, 1], I32)
            nc.gpsimd.iota(iota_col_i[:], pattern=[[0, 1]], base=0, channel_multiplier=1)
            iota_col = consts.tile([128, 1], F32)
            nc.vector.tensor_copy(iota_col[:], iota_col_i[:])
            eye64 = consts.tile([G, G], F32)
            nc.vector.tensor_scalar(out=eye64[:], in0=iota_f[0:G, :],
                                    scalar1=iota_col[0:G, :], scalar2=None,
                                    op0=ALU.is_equal)

            # long-lived similarity-phase tensors
            za_r = live.tile([128, DC, R], BF16)     # stationary for the m-slab matmuls
            invna_cols = live.tile([128, NB], F32)
            invnb_cols = live.tile([128, NB], F32)
            rsacc = live.tile([128, NB * NB], F32)   # rowsum partials, col = ib*NB+jc
            daacc = live.tile([128, NB * NB], F32)
            bs_sb = live.tile([G, G], F32)
            nc.vector.memset(bs_sb[:], 0.0)
            psg_sb = live.tile([G, 1], F32)
            v01_sb = live.tile([1, 2], F32)
            ps_cols_r = live.tile([128, NB, 2], BF16)

            # collective buffers
            warm_in = dram.tile([16], F32)
            warm_out = dram.tile([NC, 16], F32, addr_space="Shared")
            ag_in = dram.tile([DC, 128, R], BF16)
            ag_out = dram.tile([NC, DC, 128, R], BF16, addr_space="Shared")
            o1, o2, o3, o4 = G * G, G * G + G, G * G + G + 2, G * G + G + 2 + N
            CC2 = o4 + N
            cc2_in = dram.tile([CC2], F32)
            cc2_out = dram.tile([CC2], F32, addr_space="Shared")

            wz = consts.tile([1, 16], F32)
            nc.vector.memset(wz[:], 0.0)
            nc.sync.dma_start(out=warm_in[:].rearrange("(o f) -> o f", o=1), in_=wz[:])
            nc.gpsimd.collective_compute(
                "AllGather", ALU.bypass, replica_groups=[list(range(NC))],
                ins=[warm_in[:].opt()], outs=[warm_out[:].opt()])

            # ------------- Phase A: MLP (zb first; its gather overlaps za) -------------
            with tc.tile_pool(name="wpool", bufs=1) as wp, \
                 tc.tile_pool(name="mlp", bufs=1) as mp, \
                 tc.tile_pool(name="mps", bufs=2, space="PSUM") as mpp, \
                 tc.tile_pool(name="tps", bufs=2, space="PSUM") as tpp, \
                 tc.tile_pool(name="stg", bufs=1) as sg, \
                 tc.tile_pool(name="npsn", bufs=2, space="PSUM") as npn:
                w1 = wp.tile([128, HC, H], F32R)
                nc.sync.dma_start(out=w1[:], in_=w1_d[:].rearrange("(c p) o -> p c o", p=128))
                w2 = wp.tile([128, HC, H], F32R)
                nc.sync.dma_start(out=w2[:], in_=w2_d[:].rearrange("(c p) o -> p c o", p=128))
                w3 = wp.tile([128, HC, D], F32R)
                nc.sync.dma_start(out=w3[:], in_=w3_d[:].rearrange("(c p) o -> p c o", p=128))

                def load_xT(side, x_d):
                    xT = mp.tile([128, HC, R], F32R, tag="big", bufs=4, name=f"xT{side}")
                    for rb in range(NB):
                        st = sg.tile([128, H], F32, tag="stage", bufs=3)
                        nc.sync.dma_start(out=st[:], in_=x_d[rb * 128:(rb + 1) * 128, :])
                        tp4 = tpp.tile([128, H], F32, tag="tp")
                        for c in range(HC):
                            nc.tensor.transpose(tp4[:, c * 128:(c + 1) * 128],
                                                st[:, c * 128:(c + 1) * 128], ident[:])
                        nc.vector.tensor_copy(
                            xT[:, :, rb * 128:(rb + 1) * 128],
                            tp4[:].rearrange("p (c q) -> p c q", c=HC))
                    return xT

                def layer(x_in, w, n_ob, act_fn, scale, bias_cols, gc, bec, out_tag,
                          out_dtype):
                    bufs = 4 if out_tag == "big" else 1
                    out_t = mp.tile([128, n_ob, R], out_dtype, tag=out_tag, bufs=bufs,
                                    name=f"L{out_tag}{n_ob}")
                    for ob in range(n_ob):
                        ps = mpp.tile([128, R], F32, tag="mm")
                        for ns in range(NS):
                            sl = slice(ns * 512, (ns + 1) * 512)
                            for c in range(HC):
                                nc.tensor.matmul(ps[:, sl],
                                                 w[:, c, ob * 128:(ob + 1) * 128],
                                                 x_in[:, c, sl], start=(c == 0),
                                                 stop=(c == HC - 1))
                        if gc is not None:
                            th = sg.tile([128, R], F32, tag="th", bufs=2)
                            nc.scalar.activation(th[:], ps[:], act_fn, scale=scale,
                                                 bias=bias_cols[:, ob:ob + 1])
                            nc.vector.tensor_scalar(out=out_t[:, ob, :], in0=th[:],
                                                    scalar1=gc[:, ob:ob + 1],
                                                    scalar2=bec[:, ob:ob + 1],
                                                    op0=ALU.mult, op1=ALU.add)
                        else:
                            nc.scalar.activation(out_t[:, ob, :], ps[:], act_fn,
                                                 scale=scale,
                                                 bias=bias_cols[:, ob:ob + 1])
                    return out_t

                zpT = {}
                for side, x_d in (("b", zb_d), ("a", za_d)):
                    xT = load_xT(side, x_d)
                    h1 = layer(xT, w1, HC, AF.Tanh, a1_bc, ab1, g1c, be1c, "big", F32R)
                    h2 = layer(h1, w2, HC, AF.Tanh, a2_bc, ab2, g2c, be2c, "big", F32R)
                    zpT[side] = layer(h2, w3, DC, AF.Silu, 1.0, b3c, None, None,
                                      f"zp{side}", F32)

                    sq = sg.tile([128, DC, R], F32R, tag="sq")
                    nc.vector.tensor_mul(sq[:], zpT[side][:], zpT[side][:])
                    lnr = sg.tile([1, R], F32, tag="lnr")
                    for ns in range(NS):
                        sl = slice(ns * 512, (ns + 1) * 512)
                        ns2 = npn.tile([1, 512], F32, tag="smallps", name="ns2")
                        for c in range(DC):
                            nc.tensor.matmul(ns2[:], ones_k[:], sq[:, c, sl],
                                             start=(c == 0), stop=(c == DC - 1))
                        nc.scalar.activation(lnr[:, sl], ns2[:], AF.Ln)
                    if side == "b":
                        # zb_hat = zb_pT * (invnb/tau)[r]; gather it (the only gather)
                        invnt = sg.tile([1, R], F32, tag="invnt")
                        nc.scalar.activation(invnt[:], lnr[:], AF.Exp, scale=-0.5,
                                             bias=ln2_t)
                        bc = sg.tile([128, R], F32, tag="bc")
                        nc.gpsimd.partition_broadcast(bc[:], invnt[:])
                        hat = sg.tile([128, DC, R], BF16, tag="hat")
                        for c in range(DC):
                            nc.vector.tensor_mul(hat[:, c, :], zpT[side][:, c, :], bc[:])
                        nc.sync.dma_start(out=ag_in[:].rearrange("c p r -> p c r"),
                                          in_=hat[:])
                        nc.gpsimd.collective_compute(
                            "AllGather", ALU.bypass, replica_groups=[list(range(NC))],
                            ins=[ag_in[:].opt()], outs=[ag_out[:].opt()])
                        # invnb cols (for the pos_sim diag path): invnb = invnt/2
                        dnormb = dram.tile([1, R], F32, tag="dnormb")
                        nc.sync.dma_start(out=dnormb[:], in_=invnt[:])
                        ivb = sg.tile([128, NB], F32, tag="ivb")
                        nc.sync.dma_start(
                            out=ivb[:],
                            in_=dnormb[0:1, :].rearrange("o (b p) -> (o p) b", b=NB))
                        nc.vector.tensor_scalar(out=invnb_cols[:], in0=ivb[:],
                                                scalar1=float(TAU), scalar2=None,
                                                op0=ALU.mult)
                    else:
                        # za stays raw; invna applied at exp time (per-partition scale)
                        invn = sg.tile([1, R], F32, tag="invnt")
                        nc.scalar.activation(invn[:], lnr[:], AF.Exp, scale=-0.5)
                        dnorma = dram.tile([1, R], F32, tag="dnorma")
                        nc.sync.dma_start(out=dnorma[:], in_=invn[:])
                        nc.sync.dma_start(
                            out=invna_cols[:],
                            in_=dnorma[0:1, :].rearrange("o (b p) -> (o p) b", b=NB))

                nc.vector.tensor_copy(za_r[:], zpT["a"][:])

                # output slab: transpose zpT back to natural and store
                for rb in range(NB):
                    tp4 = tpp.tile([128, 2 * D], F32, tag="tp")
                    for k, side in enumerate(("a", "b")):
                        for ob in range(DC):
                            nc.tensor.transpose(
                                tp4[:, k * D + ob * 128:k * D + (ob + 1) * 128],
                                zpT[side][:, ob, rb * 128:(rb + 1) * 128], ident[:])
                    ost = sg.tile([128, 2 * D], F32, tag="ost", bufs=2)
                    nc.vector.tensor_copy(ost[:], tp4[:])
                    nc.sync.dma_start(out=out_d[rb * 128:(rb + 1) * 128, :], in_=ost[:])

                # pos_sim (diag of m): exp(rawdot*invna*invnb/tau); psg partial
                prod = sg.tile([128, DC, R], F32R, tag="sq")
                nc.vector.tensor_mul(prod[:], zpT["a"][:], zpT["b"][:])
                rd_ps = npn.tile([128, NB, 2], F32, tag="smallps", name="rd_ps")
                for ib in range(NB):
                    for c in range(DC):
                        nc.tensor.matmul(rd_ps[:, ib, :],
                                         prod[:, c, ib * 128:(ib + 1) * 128],
                                         ones2[:], start=(c == 0), stop=(c == DC - 1))
                t1 = sg.tile([128, NB], F32, tag="t1")
                nc.vector.tensor_mul(t1[:], rd_ps[:, :, 0], invna_cols[:])
                t2 = sg.tile([128, NB], F32, tag="t2")
                nc.vector.tensor_mul(t2[:], t1[:], invnb_cols[:])
                ps_cols = sg.tile([128, NB], F32, tag="t3")
                nc.scalar.activation(ps_cols[:], t2[:], AF.Exp, scale=float(1.0 / TAU))
                zcols = sg.tile([128, NB], F32, tag="zcols")
                nc.vector.memset(zcols[:], 0.0)
                nc.vector.tensor_copy(ps_cols_r[:, :, 1], zcols[:])
                nc.vector.tensor_copy(ps_cols_r[:, :, 0], ps_cols[:])
                psg_ps = npn.tile([G, 2], F32, tag="smallps", name="psg_ps")
                for ib in range(NB):
                    nc.tensor.matmul(psg_ps[:], hot_r[:, ib, 0:G], ps_cols_r[:, ib, :],
                                     start=(ib == 0), stop=(ib == NB - 1))
                nc.vector.tensor_copy(psg_sb[:], psg_ps[:, 0:1])

            # ---------------- Phase C: similarity slab ----------------
            with tc.tile_pool(name="strm", bufs=3) as strm, \
                 tc.tile_pool(name="pospool", bufs=3) as pp, \
                 tc.tile_pool(name="mpool", bufs=3) as mpo, \
                 tc.tile_pool(name="scr", bufs=2) as scp, \
                 tc.tile_pool(name="sps", bufs=2, space="PSUM") as sps, \
                 tc.tile_pool(name="ups", bufs=2, space="PSUM") as ups, \
                 tc.tile_pool(name="tiny", bufs=4) as tiny:
                for jc in range(NB):
                    zb_sl = strm.tile([128, DC, R], BF16, tag="zb_sl", name="zb_sl")
                    nc.sync.dma_start(out=zb_sl[:],
                                      in_=ag_out[jc].rearrange("c p r -> p c r"))
                    # u rows 0..64: [hot|ones]^T @ m  (row 64 = colsum partial);
                    # row 96: ones^T @ (m * posT) = dot_b partial
                    u_ps = ups.tile([128, R], F32, tag="u", name="u_ps")
                    for ib in range(NB):
                        ibs = slice(ib * 128, (ib + 1) * 128)
                        acol = ib * NB + jc
                        pos_t = pp.tile([128, R], BF16, tag="pos", name="pos_t")
                        nc.gpsimd.dma_start(out=pos_t[:],
                                            in_=pos_d[ibs, jc * R:(jc + 1) * R])
                        posT_t = pp.tile([128, R], BF16, tag="posT", name="posT_t")
                        nc.gpsimd.dma_start(out=posT_t[:],
                                            in_=posT_d[ibs, jc * R:(jc + 1) * R])
                        sa = sps.tile([128, R], F32, tag="S", name="sa")
                        for ns in range(NS):
                            sl = slice(ns * 512, (ns + 1) * 512)
                            for c in range(DC):
                                nc.tensor.matmul(sa[:, sl], za_r[:, c, ibs],
                                                 zb_sl[:, c, sl],
                                                 start=(c == 0), stop=(c == DC - 1))
                        m_a = mpo.tile([128, R], BF16, tag="ma", name="m_a")
                        nc.scalar.activation(m_a[:], sa[:], AF.Exp,
                                             scale=invna_cols[:, ib:ib + 1],
                                             accum_out=rsacc[:, acol:acol + 1])
                        scr_a = scp.tile([128, R], BF16, tag="scra", name="scr_a")
                        nc.vector.scalar_tensor_tensor(
                            out=scr_a[:], in0=m_a[:], scalar=1.0, in1=pos_t[:],
                            op0=ALU.mult, op1=ALU.mult,
                            accum_out=daacc[:, acol:acol + 1])
                        prod_b = scp.tile([128, R], BF16, tag="prodb", name="prod_b")
                        nc.vector.tensor_mul(prod_b[:], m_a[:], posT_t[:])
                        for ns in range(NS):
                            sl = slice(ns * 512, (ns + 1) * 512)
                            nc.tensor.matmul(u_ps[0:65, sl], hot_r[:, ib, :], m_a[:, sl],
                                             start=(ib == 0), stop=(ib == NB - 1),
                                             skip_group_check=True)
                            nc.tensor.matmul(u_ps[96:97, sl], ones_kb[:], prod_b[:, sl],
                                             start=(ib == 0), stop=(ib == NB - 1),
                                             skip_group_check=True,
                                             tile_position=(0, 96))
                    csdb_st = scp.tile([128, R], F32, tag="csdb", name="csdb_st",
                                       bufs=2)
                    nc.scalar.copy(csdb_st[64:65, :], u_ps[64:65, :])
                    nc.scalar.copy(csdb_st[96:97, :], u_ps[96:97, :])
                    nc.sync.dma_start(
                        out=cc2_in[o3 + jc * R:o3 + (jc + 1) * R].rearrange(
                            "(o f) -> o f", o=1),
                        in_=csdb_st[64:65, :])
                    nc.sync.dma_start(
                        out=cc2_in[o4 + jc * R:o4 + (jc + 1) * R].rearrange(
                            "(o f) -> o f", o=1),
                        in_=csdb_st[96:97, :])
                    for (g, lo, hi, first) in segs_by_jc[jc]:
                        if first:
                            nc.vector.reduce_sum(bs_sb[:, g:g + 1], u_ps[0:G, lo:hi],
                                                 axis=X)
                        else:
                            tmp = tiny.tile([G, 1], F32, tag="segtmp", name="segtmp")
                            nc.vector.reduce_sum(tmp[:], u_ps[0:G, lo:hi], axis=X)
                            nc.vector.tensor_add(bs_sb[:, g:g + 1], bs_sb[:, g:g + 1],
                                                 tmp[:])

            # ---------------- Phase D: local log-sums + allreduce ----------------
            with tc.tile_pool(name="ep", bufs=1) as ep, \
                 tc.tile_pool(name="eps", bufs=2, space="PSUM") as epp:
                red = ep.tile([128, 2, NB], F32)
                for k, acc in enumerate((daacc, rsacc)):
                    nc.vector.reduce_sum(red[:, k, :],
                                         acc[:].rearrange("p (ib jc) -> p ib jc", ib=NB),
                                         axis=X)
                nc.vector.tensor_scalar(out=red[:, 1, :], in0=red[:, 1, :],
                                        scalar1=EPS_G, scalar2=None, op0=ALU.add)
                la2 = ep.tile([128, 2], F32)
                lnscr = ep.tile([128, NB], F32)
                for k in range(2):
                    nc.scalar.activation(lnscr[:], red[:, k, :], AF.Ln,
                                         accum_out=la2[:, k:k + 1])
                la2r = ep.tile([128, 2], F32R)
                nc.vector.tensor_copy(la2r[:], la2[:])
                v01_ps = epp.tile([1, 2], F32)
                nc.tensor.matmul(v01_ps[:], ones_k[:], la2r[:], start=True, stop=True)
                nc.vector.tensor_copy(v01_sb[:], v01_ps[:])

                nc.sync.dma_start(out=cc2_in[0:o1].rearrange("(g h) -> g h", g=G),
                                  in_=bs_sb[:])
                nc.sync.dma_start(out=cc2_in[o1:o2].rearrange("(g o) -> g o", g=G),
                                  in_=psg_sb[:])
                nc.sync.dma_start(out=cc2_in[o2:o3].rearrange("(o f) -> o f", o=1),
                                  in_=v01_sb[:])
                nc.gpsimd.collective_compute(
                    "AllReduce", ALU.add, replica_groups=[list(range(NC))],
                    ins=[cc2_in[:].opt()], outs=[cc2_out[:].opt()])

                # ---------------- Phase E: final scalar loss ----------------
                bs_f = ep.tile([G, G], F32)
                nc.sync.dma_start(out=bs_f[:],
                                  in_=cc2_out[0:o1].rearrange("(g h) -> g h", g=G))
                psg_f = ep.tile([G, 1], F32)
                nc.sync.dma_start(out=psg_f[:],
                                  in_=cc2_out[o1:o2].rearrange("(g o) -> g o", g=G))
                # full colsum/dot_b rows -> [128, 64] col layout
                csdb = ep.tile([128, 2, G], F32)
                nc.sync.dma_start(out=csdb[:, 0, :],
                                  in_=cc2_out[o3:o4].rearrange("(b p) -> p b", p=128))
                nc.sync.dma_start(out=csdb[:, 1, :],
                                  in_=cc2_out[o4:].rearrange("(b p) -> p b", p=128))
                # v2 = sum ln(dot_b), v3 = sum ln(colsum+eps) over ALL rows (identical
                # on every core -> bypasses the allreduce)
                nc.vector.tensor_scalar(out=csdb[:, 0, :], in0=csdb[:, 0, :],
                                        scalar1=EPS_G, scalar2=None, op0=ALU.add)
                lb2 = ep.tile([128, 2], F32)
                lnscr2 = ep.tile([128, G], F32)
                nc.scalar.activation(lnscr2[:], csdb[:, 1, :], AF.Ln,
                                     accum_out=lb2[:, 0:1])
                nc.scalar.activation(lnscr2[:], csdb[:, 0, :], AF.Ln,
                                     accum_out=lb2[:, 1:2])
                lb2r = ep.tile([128, 2], F32R)
                nc.vector.tensor_copy(lb2r[:], lb2[:])
                v23_ps = epp.tile([1, 2], F32)
                nc.tensor.matmul(v23_ps[:], ones_k[:], lb2r[:], start=True, stop=True)

                L4 = ep.tile([G, 4], F32)
                nc.sync.dma_start(out=L4[:, 0:1],
                                  in_=cc2_out[o1:o2].rearrange("(g o) -> g o", g=G))
                gs = ep.tile([G, 1], F32)
                eyescr = ep.tile([G, G], F32)
                nc.vector.scalar_tensor_tensor(out=eyescr[:], in0=bs_f[:], scalar=1.0,
                                               in1=eye64[:], op0=ALU.mult, op1=ALU.mult,
                                               accum_out=gs[:])
                neg1r = ep.tile([G, 1], F32)
                nc.vector.reduce_sum(neg1r[:], bs_f[:], axis=X)
                nc.vector.scalar_tensor_tensor(out=L4[:, 2:3], in0=neg1r[:],
                                               scalar=EPS_L, in1=gs[:], op0=ALU.add,
                                               op1=ALU.subtract)
                bs_fr = ep.tile([G, G], F32R)
                nc.vector.tensor_copy(bs_fr[:], bs_f[:])
                neg0_ps = epp.tile([G, 2], F32)
                nc.tensor.matmul(neg0_ps[:], bs_fr[:], ones2[0:G, :], start=True,
                                 stop=True)
                nc.vector.scalar_tensor_tensor(out=L4[:, 1:2], in0=neg0_ps[:, 0:1],
                                               scalar=EPS_L, in1=gs[:], op0=ALU.add,
                                               op1=ALU.subtract)
                nc.vector.scalar_tensor_tensor(out=L4[:, 3:4], in0=gs[:], scalar=EPS_L,
                                               in1=psg_f[:], op0=ALU.add,
                                               op1=ALU.subtract)
                L4ln = ep.tile([G, 4], F32)
                nc.scalar.activation(L4ln[:], L4[:], AF.Ln)
                L4r = ep.tile([G, 4], F32R)
                nc.vector.tensor_copy(L4r[:], L4ln[:])
                s4_ps = epp.tile([1, 4], F32)
                nc.tensor.matmul(s4_ps[:], ones_k[0:G, :], L4r[:], start=True, stop=True)

                vrow = ep.tile([1, 8], F32)
                nc.sync.dma_start(out=vrow[:, 0:2],
                                  in_=cc2_out[o2:o3].rearrange("(o f) -> o f", o=1))
                nc.vector.tensor_copy(vrow[:, 2:4], v23_ps[:])
                nc.vector.tensor_copy(vrow[:, 4:8], s4_ps[:])
                vscr = ep.tile([1, 8], F32)
                loss_sb = ep.tile([1, 1], F32)
                nc.vector.scalar_tensor_tensor(out=vscr[:], in0=vrow[:], scalar=1.0,
                                               in1=coef_sb[:], op0=ALU.mult, op1=ALU.mult,
                                               accum_out=loss_sb[:])
                nc.sync.dma_start(out=loss_d[:], in_=loss_sb[:])

    nc.compile()
    return nc


def kernel(**inputs):
    global LAST_RESULTS
    from concourse.bass_utils import run_bass_kernel_spmd
    import ml_dtypes

    batch = np.asarray(inputs["batch"], dtype=np.int64)
    key = batch.tobytes()
    if _PROGRAM_CACHE.get("key") != key:
        _PROGRAM_CACHE["prog"] = _build_program(batch)
        _PROGRAM_CACHE["key"] = key
    nc = _PROGRAM_CACHE["prog"]

    za = np.asarray(inputs["za"], dtype=np.float32)
    zb = np.asarray(inputs["zb"], dtype=np.float32)
    pos = np.asarray(inputs["pos"], dtype=np.float32)
    bf16 = ml_dtypes.bfloat16
    pos_bf = pos.astype(bf16)
    posT_bf = np.ascontiguousarray(pos.T).astype(bf16)
    hot = np.zeros((N, 65), dtype=bf16)
    hot[np.arange(N), batch] = 1
    hot[:, 64] = 1

    def cols(v, nb):
        return np.asarray(v, dtype=np.float32).reshape(nb, 128).T

    prm = np.zeros((128, 32), dtype=np.float32)
    prm[:, 0:4] = cols(inputs["b1"], 4)
    prm[:, 4:8] = cols(inputs["g1"], 4)
    prm[:, 8:12] = cols(inputs["be1"], 4)
    prm[:, 12:16] = cols(inputs["b2"], 4)
    prm[:, 16:20] = cols(inputs["g2"], 4)
    prm[:, 20:24] = cols(inputs["be2"], 4)
    prm[:, 24:26] = cols(inputs["b3"], 2)
    prm[:, 26] = np.float32(np.asarray(inputs["a1"]).reshape(-1)[0])
    prm[:, 27] = np.float32(np.asarray(inputs["a2"]).reshape(-1)[0])
    prm[:, 28] = np.float32(np.log(1.0 / TAU))

    coef = np.array([[-LAM / N, LAM / N, -(1.0 - LAM) / N, (1.0 - LAM) / N,
                      ALPHA / G - BETA / G, -ALPHA / (2 * G), -ALPHA / (2 * G),
                      BETA / G]], dtype=np.float32)

    shared = {
        "W1": np.asarray(inputs["W1"], dtype=np.float32),
        "W2": np.asarray(inputs["W2"], dtype=np.float32),
        "W3": np.asarray(inputs["W3"], dtype=np.float32),
        "prm": prm, "coef": coef,
    }
    in_maps = []
    for c in range(NC):
        sl = slice(c * R, (c + 1) * R)
        m = dict(shared)
        m["za_s"] = za[sl]
        m["zb_s"] = zb[sl]
        m["pos_s"] = pos_bf[sl]
        m["posT_s"] = posT_bf[sl]
        m["hot_s"] = hot[sl]
        in_maps.append(m)

    res = run_bass_kernel_spmd(nc, in_maps, list(range(NC)))
    LAST_RESULTS = res
    out = np.concatenate([res.results[c]["out_s"] for c in range(NC)], axis=0)
    loss = np.float32(res.results[0]["loss"][0, 0])
    return loss, out


# revision 27
# speedup vs baseline: 1.0848x; 1.0027x over previous
"""Bass/Trainium2 kernel for nn_EnhancedContrast (8-core SPMD).

Sharding: rows (N=8192) split across 8 cores, 1024 rows each. Each core:
  - runs the projection MLP on its za/zb row-slab (activation-transposed layout,
    zb first so the single all-gather of normalized zb overlaps za's MLP),
  - computes its row-slab of m = exp(cos/tau): rowsum/dot_a reduce locally along
    the free dim; colsum partials ride a ones-column on the one-hot matmul and
    dot_b partials come from a host-transposed bf16 pos^T via a ones-matmul
    partition reduction,
  - one AllReduce (~84KB) combines batch_sim/pos_sim_graph/colsum/dot_b/log
    partials; every core then computes the identical scalar loss.
"""
import sys

sys.path.insert(0, "/opt/trn_rl_repo")

import numpy as np

N, H, D, G = 8192, 512, 256, 64
TAU, LAM, ALPHA, BETA = 0.5, 0.5, 1.0, 1.0
EPS_G, EPS_L = 1e-6, 1e-5
NC = 8            # cores
R = N // NC       # rows per core = 1024
NB = R // 128     # 128-row blocks per core = 8
HC = H // 128     # hidden chunks = 4
DC = D // 128     # proj-dim chunks = 2
NS = R // 512     # 512-wide moving slices per 1024 = 2

LAST_RESULTS = None  # stashed BassKernelResults for test.py
_PROGRAM_CACHE = {}


def _build_program(batch_np):
    import concourse.mybir as mybir
    import concourse.tile as tile
    from concourse import bacc
    from concourse.masks import make_identity

    F32 = mybir.dt.float32
    F32R = mybir.dt.float32r
    BF16 = mybir.dt.bfloat16
    I32 = mybir.dt.int32
    AF = mybir.ActivationFunctionType
    ALU = mybir.AluOpType
    X = mybir.AxisListType.X

    # group segments along the full column axis (batch is sorted)
    bounds = np.searchsorted(batch_np, np.arange(G + 1))
    segs_by_jc = [[] for _ in range(NB)]
    for g in range(G):
        lo, hi = int(bounds[g]), int(bounds[g + 1])
        first = True
        j = lo
        while j < hi:
            jc = j // R
            e = min(hi, (jc + 1) * R)
            segs_by_jc[jc].append((g, j - jc * R, e - jc * R, first))
            first = False
            j = e

    nc = bacc.Bacc("TRN2", target_bir_lowering=False, debug=False, num_devices=NC)

    # ---- I/O ----
    za_d = nc.dram_tensor("za_s", [R, H], F32, kind="ExternalInput")
    zb_d = nc.dram_tensor("zb_s", [R, H], F32, kind="ExternalInput")
    pos_d = nc.dram_tensor("pos_s", [R, N], BF16, kind="ExternalInput")
    posT_d = nc.dram_tensor("posT_s", [R, N], BF16, kind="ExternalInput")
    hot_d = nc.dram_tensor("hot_s", [R, 65], BF16, kind="ExternalInput")
    w1_d = nc.dram_tensor("W1", [H, H], F32R, kind="ExternalInput")
    w2_d = nc.dram_tensor("W2", [H, H], F32R, kind="ExternalInput")
    w3_d = nc.dram_tensor("W3", [H, D], F32R, kind="ExternalInput")
    prm_d = nc.dram_tensor("prm", [128, 32], F32, kind="ExternalInput")
    coef_d = nc.dram_tensor("coef", [1, 8], F32, kind="ExternalInput")
    out_d = nc.dram_tensor("out_s", [R, 2 * D], F32, kind="ExternalOutput")
    loss_d = nc.dram_tensor("loss", [1, 1], F32, kind="ExternalOutput")

    with tile.TileContext(nc) as tc:
        with tc.tile_pool(name="consts", bufs=1) as consts, \
             tc.tile_pool(name="live", bufs=1) as live, \
             tc.tile_pool(name="dram", bufs=1, space="DRAM") as dram:
            # packed per-partition params: b1c g1c be1c b2c g2c be2c (4 cols each),
            # b3c (2), a1, a2, ln2 -> 29 cols used
            prm = consts.tile([128, 32], F32)
            nc.sync.dma_start(out=prm[:], in_=prm_d[:])
            b1c, g1c, be1c = prm[:, 0:4], prm[:, 4:8], prm[:, 8:12]
            b2c, g2c, be2c = prm[:, 12:16], prm[:, 16:20], prm[:, 20:24]
            b3c = prm[:, 24:26]
            a1_bc, a2_bc = prm[:, 26:27], prm[:, 27:28]
            ln2_t = prm[0:1, 28:29]
            coef_sb = consts.tile([1, 8], F32)
            nc.sync.dma_start(out=coef_sb[:], in_=coef_d[:])

            ident = consts.tile([128, 128], F32)
            make_identity(nc, ident[:])
            ones_k32 = consts.tile([128, 1], F32)
            nc.vector.memset(ones_k32[:], 1.0)
            ones_k = consts.tile([128, 1], F32R)
            nc.vector.tensor_copy(ones_k[:], ones_k32[:])
            ones_kb = consts.tile([128, 1], BF16)
            nc.vector.tensor_copy(ones_kb[:], ones_k32[:])
            ones2_32 = consts.tile([128, 2], F32)
            nc.vector.memset(ones2_32[:], 1.0)
            ones2 = consts.tile([128, 2], F32R)
            nc.vector.tensor_copy(ones2[:], ones2_32[:])

            ab1 = consts.tile([128, HC], F32)
            nc.vector.tensor_scalar(out=ab1[:], in0=b1c, scalar1=a1_bc, scalar2=None,
                                    op0=ALU.mult)
            ab2 = consts.tile([128, HC], F32)
            nc.vector.tensor_scalar(out=ab2[:], in0=b2c, scalar1=a2_bc, scalar2=None,
                                    op0=ALU.mult)

            # one-hot (64 groups + ones column) per row-block, host-prepared
            hot_r = consts.tile([128, NB, 65], BF16)
            nc.sync.dma_start(out=hot_r[:],
                              in_=hot_d[:].rearrange("(b p) g -> p b g", p=128))

            # eye64 for diag extraction in the epilogue
            iota_i = consts.tile([1, G], I32)
            nc.gpsimd.iota(iota_i[:], pattern=[[1, G]], base=0, channel_multiplier=0)
            iota_bc_i = consts.tile([128, G], I32)
            nc.gpsimd.partition_broadcast(iota_bc_i[:], iota_i[:])
            iota_f = consts.tile([128, G], F32)
            nc.vector.tensor_copy(iota_f[:], iota_bc_i[:])
            iota_col_i = consts.tile([128, 1], I32)
            nc.gpsimd.iota(iota_col_i[:], pattern=[[0, 1]], base=0, channel_multiplier=1)
            iota_col = consts.tile([128, 1], F32)
            nc.vector.tensor_copy(iota_col[:], iota_col_i[:])
            eye64 = consts.tile([G, G], F32)
            nc.vector.tensor_scalar(out=eye64[:], in0=iota_f[0:G, :],
                                    scalar1=iota_col[0:G, :], scalar2=None,
                                    op0=ALU.is_equal)

            # long-lived similarity-phase tensors
            za_r = live.tile([128, DC, R], BF16)     # stationary for the m-slab matmuls
            invna_cols = live.tile([128, NB], F32)
            invnb_cols = live.tile([128, NB], F32)
            rsacc = live.tile([128, NB * NB], F32)   # rowsum partials, col = ib*NB+jc
            daacc = live.tile([128, NB * NB], F32)
            bs_sb = live.tile([G, G], F32)
            nc.vector.memset(bs_sb[:], 0.0)
            psg_sb = live.tile([G, 1], F32)
            v01_sb = live.tile([1, 2], F32)
            ps_cols_r = live.tile([128, NB, 2], BF16)

            # collective buffers
            warm_in = dram.tile([16], F32)
            warm_out = dram.tile([NC, 16], F32, addr_space="Shared")
            ag_in = dram.tile([DC, 128, R], BF16)
            ag_out = dram.tile([NC, DC, 128, R], BF16, addr_space="Shared")
            # early allreduce: cs/db partial rows for jc 0..6 (overlaps the tail
            # of phase C); final allreduce: bs, psg, v01, cs/db for jc 7
            cca_in = dram.tile([2, NB - 1, R], F32)
            cca_out = dram.tile([2, NB - 1, R], F32, addr_space="Shared")
            o1, o2, o3, o4 = G * G, G * G + G, G * G + G + 2, G * G + G + 2 + R
            CC2 = o4 + R
            cc2_in = dram.tile([CC2], F32)
            cc2_out = dram.tile([CC2], F32, addr_space="Shared")

            wz = consts.tile([1, 16], F32)
            nc.vector.memset(wz[:], 0.0)
            nc.sync.dma_start(out=warm_in[:].rearrange("(o f) -> o f", o=1), in_=wz[:])
            nc.gpsimd.collective_compute(
                "AllGather", ALU.bypass, replica_groups=[list(range(NC))],
                ins=[warm_in[:].opt()], outs=[warm_out[:].opt()])

            # ------------- Phase A: MLP (zb first; its gather overlaps za) -------------
            with tc.tile_pool(name="wpool", bufs=1) as wp, \
                 tc.tile_pool(name="mlp", bufs=1) as mp, \
                 tc.tile_pool(name="mps", bufs=2, space="PSUM") as mpp, \
                 tc.tile_pool(name="tps", bufs=2, space="PSUM") as tpp, \
                 tc.tile_pool(name="stg", bufs=1) as sg, \
                 tc.tile_pool(name="npsn", bufs=2, space="PSUM") as npn:
                w1 = wp.tile([128, HC, H], F32R)
                nc.sync.dma_start(out=w1[:], in_=w1_d[:].rearrange("(c p) o -> p c o", p=128))
                w2 = wp.tile([128, HC, H], F32R)
                nc.sync.dma_start(out=w2[:], in_=w2_d[:].rearrange("(c p) o -> p c o", p=128))
                w3 = wp.tile([128, HC, D], F32R)
                nc.sync.dma_start(out=w3[:], in_=w3_d[:].rearrange("(c p) o -> p c o", p=128))

                def load_xT(side, x_d):
                    xT = mp.tile([128, HC, R], F32R, tag="big", bufs=4, name=f"xT{side}")
                    for rb in range(NB):
                        st = sg.tile([128, H], F32, tag="stage", bufs=3)
                        nc.sync.dma_start(out=st[:], in_=x_d[rb * 128:(rb + 1) * 128, :])
                        tp4 = tpp.tile([128, H], F32, tag="tp")
                        for c in range(HC):
                            nc.tensor.transpose(tp4[:, c * 128:(c + 1) * 128],
                                                st[:, c * 128:(c + 1) * 128], ident[:])
                        nc.vector.tensor_copy(
                            xT[:, :, rb * 128:(rb + 1) * 128],
                            tp4[:].rearrange("p (c q) -> p c q", c=HC))
                    return xT

                def layer(x_in, w, n_ob, act_fn, scale, bias_cols, gc, bec, out_tag,
                          out_dtype):
                    bufs = 4 if out_tag == "big" else 1
                    out_t = mp.tile([128, n_ob, R], out_dtype, tag=out_tag, bufs=bufs,
                                    name=f"L{out_tag}{n_ob}")
                    for ob in range(n_ob):
                        ps = mpp.tile([128, R], F32, tag="mm")
                        for ns in range(NS):
                            sl = slice(ns * 512, (ns + 1) * 512)
                            for c in range(HC):
                                nc.tensor.matmul(ps[:, sl],
                                                 w[:, c, ob * 128:(ob + 1) * 128],
                                                 x_in[:, c, sl], start=(c == 0),
                                                 stop=(c == HC - 1))
                        if gc is not None:
                            th = sg.tile([128, R], F32, tag="th", bufs=2)
                            nc.scalar.activation(th[:], ps[:], act_fn, scale=scale,
                                                 bias=bias_cols[:, ob:ob + 1])
                            nc.vector.tensor_scalar(out=out_t[:, ob, :], in0=th[:],
                                                    scalar1=gc[:, ob:ob + 1],
                                                    scalar2=bec[:, ob:ob + 1],
                                                    op0=ALU.mult, op1=ALU.add)
                        else:
                            nc.scalar.activation(out_t[:, ob, :], ps[:], act_fn,
                                                 scale=scale,
                                                 bias=bias_cols[:, ob:ob + 1])
                    return out_t

                zpT = {}
                for side, x_d in (("b", zb_d), ("a", za_d)):
                    xT = load_xT(side, x_d)
                    h1 = layer(xT, w1, HC, AF.Tanh, a1_bc, ab1, g1c, be1c, "big", F32R)
                    h2 = layer(h1, w2, HC, AF.Tanh, a2_bc, ab2, g2c, be2c, "big", F32R)
                    zpT[side] = layer(h2, w3, DC, AF.Silu, 1.0, b3c, None, None,
                                      f"zp{side}", F32)

                    sq = sg.tile([128, DC, R], F32R, tag="sq")
                    nc.vector.tensor_mul(sq[:], zpT[side][:], zpT[side][:])
                    lnr = sg.tile([1, R], F32, tag="lnr")
                    for ns in range(NS):
                        sl = slice(ns * 512, (ns + 1) * 512)
                        ns2 = npn.tile([1, 512], F32, tag="smallps", name="ns2")
                        for c in range(DC):
                            nc.tensor.matmul(ns2[:], ones_k[:], sq[:, c, sl],
                                             start=(c == 0), stop=(c == DC - 1))
                        nc.scalar.activation(lnr[:, sl], ns2[:], AF.Ln)
                    if side == "b":
                        # zb_hat = zb_pT * (invnb/tau)[r]; gather it (the only gather)
                        invnt = sg.tile([1, R], F32, tag="invnt")
                        nc.scalar.activation(invnt[:], lnr[:], AF.Exp, scale=-0.5,
                                             bias=ln2_t)
                        bc = sg.tile([128, R], F32, tag="bc")
                        nc.gpsimd.partition_broadcast(bc[:], invnt[:])
                        hat = sg.tile([128, DC, R], BF16, tag="hat")
                        for c in range(DC):
                            nc.vector.tensor_mul(hat[:, c, :], zpT[side][:, c, :], bc[:])
                        nc.sync.dma_start(out=ag_in[:].rearrange("c p r -> p c r"),
                                          in_=hat[:])
                        nc.gpsimd.collective_compute(
                            "AllGather", ALU.bypass, replica_groups=[list(range(NC))],
                            ins=[ag_in[:].opt()], outs=[ag_out[:].opt()])
                        # invnb cols (for the pos_sim diag path): invnb = invnt/2
                        dnormb = dram.tile([1, R], F32, tag="dnormb")
                        nc.sync.dma_start(out=dnormb[:], in_=invnt[:])
                        ivb = sg.tile([128, NB], F32, tag="ivb")
                        nc.sync.dma_start(
                            out=ivb[:],
                            in_=dnormb[0:1, :].rearrange("o (b p) -> (o p) b", b=NB))
                        nc.vector.tensor_scalar(out=invnb_cols[:], in0=ivb[:],
                                                scalar1=float(TAU), scalar2=None,
                                                op0=ALU.mult)
                    else:
                        # za stays raw; invna applied at exp time (per-partition scale)
                        invn = sg.tile([1, R], F32, tag="invnt")
                        nc.scalar.activation(invn[:], lnr[:], AF.Exp, scale=-0.5)
                        dnorma = dram.tile([1, R], F32, tag="dnorma")
                        nc.sync.dma_start(out=dnorma[:], in_=invn[:])
                        nc.sync.dma_start(
                            out=invna_cols[:],
                            in_=dnorma[0:1, :].rearrange("o (b p) -> (o p) b", b=NB))

                nc.vector.tensor_copy(za_r[:], zpT["a"][:])

                # output slab: transpose zpT back to natural and store
                for rb in range(NB):
                    tp4 = tpp.tile([128, 2 * D], F32, tag="tp")
                    for k, side in enumerate(("a", "b")):
                        for ob in range(DC):
                            nc.tensor.transpose(
                                tp4[:, k * D + ob * 128:k * D + (ob + 1) * 128],
                                zpT[side][:, ob, rb * 128:(rb + 1) * 128], ident[:])
                    ost = sg.tile([128, 2 * D], F32, tag="ost", bufs=2)
                    nc.vector.tensor_copy(ost[:], tp4[:])
                    nc.sync.dma_start(out=out_d[rb * 128:(rb + 1) * 128, :], in_=ost[:])

                # pos_sim (diag of m): exp(rawdot*invna*invnb/tau); psg partial
                prod = sg.tile([128, DC, R], F32R, tag="sq")
                nc.vector.tensor_mul(prod[:], zpT["a"][:], zpT["b"][:])
                rd_ps = npn.tile([128, NB, 2], F32, tag="smallps", name="rd_ps")
                for ib in range(NB):
                    for c in range(DC):
                        nc.tensor.matmul(rd_ps[:, ib, :],
                                         prod[:, c, ib * 128:(ib + 1) * 128],
                                         ones2[:], start=(c == 0), stop=(c == DC - 1))
                t1 = sg.tile([128, NB], F32, tag="t1")
                nc.vector.tensor_mul(t1[:], rd_ps[:, :, 0], invna_cols[:])
                t2 = sg.tile([128, NB], F32, tag="t2")
                nc.vector.tensor_mul(t2[:], t1[:], invnb_cols[:])
                ps_cols = sg.tile([128, NB], F32, tag="t3")
                nc.scalar.activation(ps_cols[:], t2[:], AF.Exp, scale=float(1.0 / TAU))
                zcols = sg.tile([128, NB], F32, tag="zcols")
                nc.vector.memset(zcols[:], 0.0)
                nc.vector.tensor_copy(ps_cols_r[:, :, 1], zcols[:])
                nc.vector.tensor_copy(ps_cols_r[:, :, 0], ps_cols[:])
                psg_ps = npn.tile([G, 2], F32, tag="smallps", name="psg_ps")
                for ib in range(NB):
                    nc.tensor.matmul(psg_ps[:], hot_r[:, ib, 0:G], ps_cols_r[:, ib, :],
                                     start=(ib == 0), stop=(ib == NB - 1))
                nc.vector.tensor_copy(psg_sb[:], psg_ps[:, 0:1])

            # ---------------- Phase C: similarity slab ----------------
            with tc.tile_pool(name="strm", bufs=3) as strm, \
                 tc.tile_pool(name="pospool", bufs=3) as pp, \
                 tc.tile_pool(name="mpool", bufs=3) as mpo, \
                 tc.tile_pool(name="scr", bufs=2) as scp, \
                 tc.tile_pool(name="sps", bufs=2, space="PSUM") as sps, \
                 tc.tile_pool(name="ups", bufs=2, space="PSUM") as ups, \
                 tc.tile_pool(name="tiny", bufs=4) as tiny:
                for jc in range(NB):
                    zb_sl = strm.tile([128, DC, R], BF16, tag="zb_sl", name="zb_sl")
                    nc.sync.dma_start(out=zb_sl[:],
                                      in_=ag_out[jc].rearrange("c p r -> p c r"))
                    # u rows 0..64: [hot|ones]^T @ m  (row 64 = colsum partial);
                    # row 96: ones^T @ (m * posT) = dot_b partial
                    u_ps = ups.tile([128, R], F32, tag="u", name="u_ps")
                    for ib in range(NB):
                        ibs = slice(ib * 128, (ib + 1) * 128)
                        acol = ib * NB + jc
                        pos_t = pp.tile([128, R], BF16, tag="pos", name="pos_t")
                        nc.gpsimd.dma_start(out=pos_t[:],
                                            in_=pos_d[ibs, jc * R:(jc + 1) * R])
                        posT_t = pp.tile([128, R], BF16, tag="posT", name="posT_t")
                        nc.gpsimd.dma_start(out=posT_t[:],
                                            in_=posT_d[ibs, jc * R:(jc + 1) * R])
                        sa = sps.tile([128, R], F32, tag="S", name="sa")
                        for ns in range(NS):
                            sl = slice(ns * 512, (ns + 1) * 512)
                            for c in range(DC):
                                nc.tensor.matmul(sa[:, sl], za_r[:, c, ibs],
                                                 zb_sl[:, c, sl],
                                                 start=(c == 0), stop=(c == DC - 1))
                        m_a = mpo.tile([128, R], BF16, tag="ma", name="m_a")
                        nc.scalar.activation(m_a[:], sa[:], AF.Exp,
                                             scale=invna_cols[:, ib:ib + 1],
                                             accum_out=rsacc[:, acol:acol + 1])
                        scr_a = scp.tile([128, R], BF16, tag="scra", name="scr_a")
                        nc.vector.scalar_tensor_tensor(
                            out=scr_a[:], in0=m_a[:], scalar=1.0, in1=pos_t[:],
                            op0=ALU.mult, op1=ALU.mult,
                            accum_out=daacc[:, acol:acol + 1])
                        prod_b = scp.tile([128, R], BF16, tag="prodb", name="prod_b")
                        nc.vector.tensor_mul(prod_b[:], m_a[:], posT_t[:])
                        for ns in range(NS):
                            sl = slice(ns * 512, (ns + 1) * 512)
                            nc.tensor.matmul(u_ps[0:65, sl], hot_r[:, ib, :], m_a[:, sl],
                                             start=(ib == 0), stop=(ib == NB - 1),
                                             skip_group_check=True)
                            nc.tensor.matmul(u_ps[96:97, sl], ones_kb[:], prod_b[:, sl],
                                             start=(ib == 0), stop=(ib == NB - 1),
                                             skip_group_check=True,
                                             tile_position=(0, 96))
                    csdb_st = scp.tile([128, R], F32, tag="csdb", name="csdb_st",
                                       bufs=2)
                    nc.scalar.copy(csdb_st[64:65, :], u_ps[64:65, :])
                    nc.scalar.copy(csdb_st[96:97, :], u_ps[96:97, :])
                    if jc < NB - 1:
                        nc.sync.dma_start(out=cca_in[0, jc].rearrange("(o f) -> o f", o=1),
                                          in_=csdb_st[64:65, :])
                        nc.sync.dma_start(out=cca_in[1, jc].rearrange("(o f) -> o f", o=1),
                                          in_=csdb_st[96:97, :])
                    else:
                        nc.sync.dma_start(
                            out=cc2_in[o3:o4].rearrange("(o f) -> o f", o=1),
                            in_=csdb_st[64:65, :])
                        nc.sync.dma_start(
                            out=cc2_in[o4:].rearrange("(o f) -> o f", o=1),
                            in_=csdb_st[96:97, :])
                    if jc == NB - 2:
                        nc.gpsimd.collective_compute(
                            "AllReduce", ALU.add, replica_groups=[list(range(NC))],
                            ins=[cca_in[:].opt()], outs=[cca_out[:].opt()])
                    for (g, lo, hi, first) in segs_by_jc[jc]:
                        if first:
                            nc.vector.reduce_sum(bs_sb[:, g:g + 1], u_ps[0:G, lo:hi],
                                                 axis=X)
                        else:
                            tmp = tiny.tile([G, 1], F32, tag="segtmp", name="segtmp")
                            nc.vector.reduce_sum(tmp[:], u_ps[0:G, lo:hi], axis=X)
                            nc.vector.tensor_add(bs_sb[:, g:g + 1], bs_sb[:, g:g + 1],
                                                 tmp[:])

            # ---------------- Phase D: local log-sums + allreduce ----------------
            with tc.tile_pool(name="ep", bufs=1) as ep, \
                 tc.tile_pool(name="eps", bufs=2, space="PSUM") as epp:
                red = ep.tile([128, 2, NB], F32)
                for k, acc in enumerate((daacc, rsacc)):
                    nc.vector.reduce_sum(red[:, k, :],
                                         acc[:].rearrange("p (ib jc) -> p ib jc", ib=NB),
                                         axis=X)
                nc.vector.tensor_scalar(out=red[:, 1, :], in0=red[:, 1, :],
                                        scalar1=EPS_G, scalar2=None, op0=ALU.add)
                la2 = ep.tile([128, 2], F32)
                lnscr = ep.tile([128, NB], F32)
                for k in range(2):
                    nc.scalar.activation(lnscr[:], red[:, k, :], AF.Ln,
                                         accum_out=la2[:, k:k + 1])
                la2r = ep.tile([128, 2], F32R)
                nc.vector.tensor_copy(la2r[:], la2[:])
                v01_ps = epp.tile([1, 2], F32)
                nc.tensor.matmul(v01_ps[:], ones_k[:], la2r[:], start=True, stop=True)
                nc.vector.tensor_copy(v01_sb[:], v01_ps[:])

                nc.sync.dma_start(out=cc2_in[0:o1].rearrange("(g h) -> g h", g=G),
                                  in_=bs_sb[:])
                nc.sync.dma_start(out=cc2_in[o1:o2].rearrange("(g o) -> g o", g=G),
                                  in_=psg_sb[:])
                nc.sync.dma_start(out=cc2_in[o2:o3].rearrange("(o f) -> o f", o=1),
                                  in_=v01_sb[:])
                nc.gpsimd.collective_compute(
                    "AllReduce", ALU.add, replica_groups=[list(range(NC))],
                    ins=[cc2_in[:].opt()], outs=[cc2_out[:].opt()])

                # ---------------- Phase E: final scalar loss ----------------
                bs_f = ep.tile([G, G], F32)
                nc.sync.dma_start(out=bs_f[:],
                                  in_=cc2_out[0:o1].rearrange("(g h) -> g h", g=G))
                psg_f = ep.tile([G, 1], F32)
                nc.sync.dma_start(out=psg_f[:],
                                  in_=cc2_out[o1:o2].rearrange("(g o) -> g o", g=G))
                # full colsum/dot_b rows -> [128, 64] col layout (jc<7 from the
                # early allreduce, jc=7 from the final one)
                csdb = ep.tile([128, 2, G], F32)
                for k in range(2):
                    nc.sync.dma_start(
                        out=csdb[:, k, 0:(NB - 1) * NB],
                        in_=cca_out[k].rearrange("j (b p) -> p (j b)", p=128))
                src7 = cc2_out[o3:o4] if True else None
                nc.sync.dma_start(
                    out=csdb[:, 0, (NB - 1) * NB:],
                    in_=cc2_out[o3:o4].rearrange("(b p) -> p b", p=128))
                nc.sync.dma_start(
                    out=csdb[:, 1, (NB - 1) * NB:],
                    in_=cc2_out[o4:].rearrange("(b p) -> p b", p=128))
                # v2 = sum ln(dot_b), v3 = sum ln(colsum+eps) over ALL rows (identical
                # on every core -> bypasses the allreduce)
                nc.vector.tensor_scalar(out=csdb[:, 0, :], in0=csdb[:, 0, :],
                                        scalar1=EPS_G, scalar2=None, op0=ALU.add)
                lb2 = ep.tile([128, 2], F32)
                lnscr2 = ep.tile([128, G], F32)
                nc.scalar.activation(lnscr2[:], csdb[:, 1, :], AF.Ln,
                                     accum_out=lb2[:, 0:1])
                nc.scalar.activation(lnscr2[:], csdb[:, 0, :], AF.Ln,
                                     accum_out=lb2[:, 1:2])
                lb2r = ep.tile([128, 2], F32R)
                nc.vector.tensor_copy(lb2r[:], lb2[:])
                v23_ps = epp.tile([1, 2], F32)
                nc.tensor.matmul(v23_ps[:], ones_k[:], lb2r[:], start=True, stop=True)

                L4 = ep.tile([G, 4], F32)
                nc.sync.dma_start(out=L4[:, 0:1],
                                  in_=cc2_out[o1:o2].rearrange("(g o) -> g o", g=G))
                gs = ep.tile([G, 1], F32)
                eyescr = ep.tile([G, G], F32)
                nc.vector.scalar_tensor_tensor(out=eyescr[:], in0=bs_f[:], scalar=1.0,
                                               in1=eye64[:], op0=ALU.mult, op1=ALU.mult,
                                               accum_out=gs[:])
                neg1r = ep.tile([G, 1], F32)
                nc.vector.reduce_sum(neg1r[:], bs_f[:], axis=X)
                nc.vector.scalar_tensor_tensor(out=L4[:, 2:3], in0=neg1r[:],
                                               scalar=EPS_L, in1=gs[:], op0=ALU.add,
                                               op1=ALU.subtract)
                bs_fr = ep.tile([G, G], F32R)
                nc.vector.tensor_copy(bs_fr[:], bs_f[:])
                neg0_ps = epp.tile([G, 2], F32)
                nc.tensor.matmul(neg0_ps[:], bs_fr[:], ones2[0:G, :], start=True,
                                 stop=True)
                nc.vector.scalar_tensor_tensor(out=L4[:, 1:2], in0=neg0_ps[:, 0:1],
                                               scalar=EPS_L, in1=gs[:], op0=ALU.add,
                                               op1=ALU.subtract)
                nc.vector.scalar_tensor_tensor(out=L4[:, 3:4], in0=gs[:], scalar=EPS_L,
                                               in1=psg_f[:], op0=ALU.add,
                                               op1=ALU.subtract)
                L4ln = ep.tile([G, 4], F32)
                nc.scalar.activation(L4ln[:], L4[:], AF.Ln)
                L4r = ep.tile([G, 4], F32R)
                nc.vector.tensor_copy(L4r[:], L4ln[:])
                s4_ps = epp.tile([1, 4], F32)
                nc.tensor.matmul(s4_ps[:], ones_k[0:G, :], L4r[:], start=True, stop=True)

                vrow = ep.tile([1, 8], F32)
                nc.sync.dma_start(out=vrow[:, 0:2],
                                  in_=cc2_out[o2:o3].rearrange("(o f) -> o f", o=1))
                nc.vector.tensor_copy(vrow[:, 2:4], v23_ps[:])
                nc.vector.tensor_copy(vrow[:, 4:8], s4_ps[:])
                vscr = ep.tile([1, 8], F32)
                loss_sb = ep.tile([1, 1], F32)
                nc.vector.scalar_tensor_tensor(out=vscr[:], in0=vrow[:], scalar=1.0,
                                               in1=coef_sb[:], op0=ALU.mult, op1=ALU.mult,
                                               accum_out=loss_sb[:])
                nc.sync.dma_start(out=loss_d[:], in_=loss_sb[:])

    nc.compile()
    return nc


def kernel(**inputs):
    global LAST_RESULTS
    from concourse.bass_utils import run_bass_kernel_spmd
    import ml_dtypes

    batch = np.asarray(inputs["batch"], dtype=np.int64)
    key = batch.tobytes()
    if _PROGRAM_CACHE.get("key") != key:
        _PROGRAM_CACHE["prog"] = _build_program(batch)
        _PROGRAM_CACHE["key"] = key
    nc = _PROGRAM_CACHE["prog"]

    za = np.asarray(inputs["za"], dtype=np.float32)
    zb = np.asarray(inputs["zb"], dtype=np.float32)
    pos = np.asarray(inputs["pos"], dtype=np.float32)
    bf16 = ml_dtypes.bfloat16
    pos_bf = pos.astype(bf16)
    posT_bf = np.ascontiguousarray(pos.T).astype(bf16)
    hot = np.zeros((N, 65), dtype=bf16)
    hot[np.arange(N), batch] = 1
    hot[:, 64] = 1

    def cols(v, nb):
        return np.asarray(v, dtype=np.float32).reshape(nb, 128).T

    prm = np.zeros((128, 32), dtype=np.float32)
    prm[:, 0:4] = cols(inputs["b1"], 4)
    prm[:, 4:8] = cols(inputs["g1"], 4)
    prm[:, 8:12] = cols(inputs["be1"], 4)
    prm[:, 12:16] = cols(inputs["b2"], 4)
    prm[:, 16:20] = cols(inputs["g2"], 4)
    prm[:, 20:24] = cols(inputs["be2"], 4)
    prm[:, 24:26] = cols(inputs["b3"], 2)
    prm[:, 26] = np.float32(np.asarray(inputs["a1"]).reshape(-1)[0])
    prm[:, 27] = np.float32(np.asarray(inputs["a2"]).reshape(-1)[0])
    prm[:, 28] = np.float32(np.log(1.0 / TAU))

    coef = np.array([[-LAM / N, LAM / N, -(1.0 - LAM) / N, (1.0 - LAM) / N,
                      ALPHA / G - BETA / G, -ALPHA / (2 * G), -ALPHA / (2 * G),
                      BETA / G]], dtype=np.float32)

    shared = {
        "W1": np.asarray(inputs["W1"], dtype=np.float32),
        "W2": np.asarray(inputs["W2"], dtype=np.float32),
        "W3": np.asarray(inputs["W3"], dtype=np.float32),
        "prm": prm, "coef": coef,
    }
    in_maps = []
    for c in range(NC):
        sl = slice(c * R, (c + 1) * R)
        m = dict(shared)
        m["za_s"] = za[sl]
        m["zb_s"] = zb[sl]
        m["pos_s"] = pos_bf[sl]
        m["posT_s"] = posT_bf[sl]
        m["hot_s"] = hot[sl]
        in_maps.append(m)

    res = run_bass_kernel_spmd(nc, in_maps, list(range(NC)))
    LAST_RESULTS = res
    out = np.concatenate([res.results[c]["out_s"] for c in range(NC)], axis=0)
    loss = np.float32(res.results[0]["loss"][0, 0])
    return loss, out


# revision 28
# speedup vs baseline: 1.0957x; 1.0100x over previous
"""Bass/Trainium2 kernel for nn_EnhancedContrast (8-core SPMD).

Sharding: rows (N=8192) split across 8 cores, 1024 rows each. Each core:
  - runs the projection MLP on its za/zb row-slab (activation-transposed layout,
    zb first so the single all-gather of normalized zb overlaps za's MLP),
  - computes its row-slab of m = exp(cos/tau): rowsum/dot_a reduce locally along
    the free dim; colsum partials ride a ones-column on the one-hot matmul and
    dot_b partials come from a host-transposed bf16 pos^T via a ones-matmul
    partition reduction,
  - one AllReduce (~84KB) combines batch_sim/pos_sim_graph/colsum/dot_b/log
    partials; every core then computes the identical scalar loss.
"""
import sys

sys.path.insert(0, "/opt/trn_rl_repo")

import numpy as np

N, H, D, G = 8192, 512, 256, 64
TAU, LAM, ALPHA, BETA = 0.5, 0.5, 1.0, 1.0
EPS_G, EPS_L = 1e-6, 1e-5
NC = 8            # cores
R = N // NC       # rows per core = 1024
NB = R // 128     # 128-row blocks per core = 8
HC = H // 128     # hidden chunks = 4
DC = D // 128     # proj-dim chunks = 2
NS = R // 512     # 512-wide moving slices per 1024 = 2

LAST_RESULTS = None  # stashed BassKernelResults for test.py
_PROGRAM_CACHE = {}


def _build_program(batch_np):
    import concourse.mybir as mybir
    import concourse.tile as tile
    from concourse import bacc
    from concourse.masks import make_identity

    F32 = mybir.dt.float32
    F32R = mybir.dt.float32r
    BF16 = mybir.dt.bfloat16
    I32 = mybir.dt.int32
    AF = mybir.ActivationFunctionType
    ALU = mybir.AluOpType
    X = mybir.AxisListType.X

    # group segments along the full column axis (batch is sorted)
    bounds = np.searchsorted(batch_np, np.arange(G + 1))
    segs_by_jc = [[] for _ in range(NB)]
    for g in range(G):
        lo, hi = int(bounds[g]), int(bounds[g + 1])
        first = True
        j = lo
        while j < hi:
            jc = j // R
            e = min(hi, (jc + 1) * R)
            segs_by_jc[jc].append((g, j - jc * R, e - jc * R, first))
            first = False
            j = e

    nc = bacc.Bacc("TRN2", target_bir_lowering=False, debug=False, num_devices=NC)

    # ---- I/O ----
    za_d = nc.dram_tensor("za_s", [R, H], F32, kind="ExternalInput")
    zb_d = nc.dram_tensor("zb_s", [R, H], F32, kind="ExternalInput")
    pos_d = nc.dram_tensor("pos_s", [R, N], BF16, kind="ExternalInput")
    posT_d = nc.dram_tensor("posT_s", [R, N], BF16, kind="ExternalInput")
    hot_d = nc.dram_tensor("hot_s", [R, 65], BF16, kind="ExternalInput")
    w1_d = nc.dram_tensor("W1", [H, H], F32R, kind="ExternalInput")
    w2_d = nc.dram_tensor("W2", [H, H], F32R, kind="ExternalInput")
    w3_d = nc.dram_tensor("W3", [H, D], F32R, kind="ExternalInput")
    prm_d = nc.dram_tensor("prm", [128, 32], F32, kind="ExternalInput")
    coef_d = nc.dram_tensor("coef", [1, 8], F32, kind="ExternalInput")
    out_d = nc.dram_tensor("out_s", [R, 2 * D], F32, kind="ExternalOutput")
    loss_d = nc.dram_tensor("loss", [1, 1], F32, kind="ExternalOutput")

    with tile.TileContext(nc) as tc:
        with tc.tile_pool(name="consts", bufs=1) as consts, \
             tc.tile_pool(name="live", bufs=1) as live, \
             tc.tile_pool(name="dram", bufs=1, space="DRAM") as dram:
            # packed per-partition params: b1c g1c be1c b2c g2c be2c (4 cols each),
            # b3c (2), a1, a2, ln2 -> 29 cols used
            prm = consts.tile([128, 32], F32)
            nc.sync.dma_start(out=prm[:], in_=prm_d[:])
            b1c, g1c, be1c = prm[:, 0:4], prm[:, 4:8], prm[:, 8:12]
            b2c, g2c, be2c = prm[:, 12:16], prm[:, 16:20], prm[:, 20:24]
            b3c = prm[:, 24:26]
            a1_bc, a2_bc = prm[:, 26:27], prm[:, 27:28]
            ln2_t = prm[0:1, 28:29]
            coef_sb = consts.tile([1, 8], F32)
            nc.sync.dma_start(out=coef_sb[:], in_=coef_d[:])

            ident = consts.tile([128, 128], F32)
            make_identity(nc, ident[:])
            ones_k32 = consts.tile([128, 1], F32)
            nc.vector.memset(ones_k32[:], 1.0)
            ones_k = consts.tile([128, 1], F32R)
            nc.vector.tensor_copy(ones_k[:], ones_k32[:])
            ones_kb = consts.tile([128, 1], BF16)
            nc.vector.tensor_copy(ones_kb[:], ones_k32[:])
            ones2_32 = consts.tile([128, 2], F32)
            nc.vector.memset(ones2_32[:], 1.0)
            ones2 = consts.tile([128, 2], F32R)
            nc.vector.tensor_copy(ones2[:], ones2_32[:])

            ab1 = consts.tile([128, HC], F32)
            nc.vector.tensor_scalar(out=ab1[:], in0=b1c, scalar1=a1_bc, scalar2=None,
                                    op0=ALU.mult)
            ab2 = consts.tile([128, HC], F32)
            nc.vector.tensor_scalar(out=ab2[:], in0=b2c, scalar1=a2_bc, scalar2=None,
                                    op0=ALU.mult)

            # one-hot (64 groups + ones column) per row-block, host-prepared
            hot_r = consts.tile([128, NB, 65], BF16)
            nc.sync.dma_start(out=hot_r[:],
                              in_=hot_d[:].rearrange("(b p) g -> p b g", p=128))

            # eye64 for diag extraction in the epilogue
            iota_i = consts.tile([1, G], I32)
            nc.gpsimd.iota(iota_i[:], pattern=[[1, G]], base=0, channel_multiplier=0)
            iota_bc_i = consts.tile([128, G], I32)
            nc.gpsimd.partition_broadcast(iota_bc_i[:], iota_i[:])
            iota_f = consts.tile([128, G], F32)
            nc.vector.tensor_copy(iota_f[:], iota_bc_i[:])
            iota_col_i = consts.tile([128, 1], I32)
            nc.gpsimd.iota(iota_col_i[:], pattern=[[0, 1]], base=0, channel_multiplier=1)
            iota_col = consts.tile([128, 1], F32)
            nc.vector.tensor_copy(iota_col[:], iota_col_i[:])
            eye64 = consts.tile([G, G], F32)
            nc.vector.tensor_scalar(out=eye64[:], in0=iota_f[0:G, :],
                                    scalar1=iota_col[0:G, :], scalar2=None,
                                    op0=ALU.is_equal)

            # long-lived similarity-phase tensors
            za_r = live.tile([128, DC, R], BF16)     # stationary for the m-slab matmuls
            invna_cols = live.tile([128, NB], F32)
            invnb_cols = live.tile([128, NB], F32)
            rsacc = live.tile([128, NB * NB], F32)   # rowsum partials, col = ib*NB+jc
            daacc = live.tile([128, NB * NB], F32)
            bs_sb = live.tile([G, G], F32)
            nc.vector.memset(bs_sb[:], 0.0)
            psg_sb = live.tile([G, 1], F32)
            v01_sb = live.tile([1, 2], F32)
            ps_cols_r = live.tile([128, NB, 2], BF16)

            # collective buffers
            warm_in = dram.tile([16], F32)
            warm_out = dram.tile([NC, 16], F32, addr_space="Shared")
            ag_in = dram.tile([DC, 128, R], BF16)
            ag_out = dram.tile([NC, DC, 128, R], BF16, addr_space="Shared")
            # early allreduce: cs/db partial rows for jc 0..6 (overlaps the tail
            # of phase C); final allreduce: bs, psg, v01, cs/db for jc 7
            cca_in = dram.tile([2, NB - 1, R], F32)
            cca_out = dram.tile([2, NB - 1, R], F32, addr_space="Shared")
            o1, o2, o3, o4 = G * G, G * G + G, G * G + G + 2, G * G + G + 2 + R
            CC2 = o4 + R
            cc2_in = dram.tile([CC2], F32)
            cc2_out = dram.tile([CC2], F32, addr_space="Shared")

            wz = consts.tile([1, 16], F32)
            nc.vector.memset(wz[:], 0.0)
            nc.sync.dma_start(out=warm_in[:].rearrange("(o f) -> o f", o=1), in_=wz[:])
            nc.gpsimd.collective_compute(
                "AllGather", ALU.bypass, replica_groups=[list(range(NC))],
                ins=[warm_in[:].opt()], outs=[warm_out[:].opt()])

            # ------------- Phase A: MLP (zb first; its gather overlaps za) -------------
            with tc.tile_pool(name="wpool", bufs=1) as wp, \
                 tc.tile_pool(name="mlp", bufs=1) as mp, \
                 tc.tile_pool(name="mps", bufs=2, space="PSUM") as mpp, \
                 tc.tile_pool(name="tps", bufs=2, space="PSUM") as tpp, \
                 tc.tile_pool(name="stg", bufs=1) as sg, \
                 tc.tile_pool(name="npsn", bufs=2, space="PSUM") as npn:
                w1 = wp.tile([128, HC, H], F32R)
                nc.sync.dma_start(out=w1[:], in_=w1_d[:].rearrange("(c p) o -> p c o", p=128))
                w2 = wp.tile([128, HC, H], F32R)
                nc.sync.dma_start(out=w2[:], in_=w2_d[:].rearrange("(c p) o -> p c o", p=128))
                w3 = wp.tile([128, HC, D], F32R)
                nc.sync.dma_start(out=w3[:], in_=w3_d[:].rearrange("(c p) o -> p c o", p=128))

                def load_xT(side, x_d):
                    xT = mp.tile([128, HC, R], F32R, tag="big", bufs=4, name=f"xT{side}")
                    for rb in range(NB):
                        st = sg.tile([128, H], F32, tag="stage", bufs=3)
                        nc.sync.dma_start(out=st[:], in_=x_d[rb * 128:(rb + 1) * 128, :])
                        tp4 = tpp.tile([128, H], F32, tag="tp")
                        for c in range(HC):
                            nc.tensor.transpose(tp4[:, c * 128:(c + 1) * 128],
                                                st[:, c * 128:(c + 1) * 128], ident[:])
                        nc.vector.tensor_copy(
                            xT[:, :, rb * 128:(rb + 1) * 128],
                            tp4[:].rearrange("p (c q) -> p c q", c=HC))
                    return xT

                def layer(x_in, w, n_ob, act_fn, scale, bias_cols, gc, bec, out_tag,
                          out_dtype):
                    bufs = 4 if out_tag == "big" else 1
                    out_t = mp.tile([128, n_ob, R], out_dtype, tag=out_tag, bufs=bufs,
                                    name=f"L{out_tag}{n_ob}")
                    for ob in range(n_ob):
                        ps = mpp.tile([128, R], F32, tag="mm")
                        for ns in range(NS):
                            sl = slice(ns * 512, (ns + 1) * 512)
                            for c in range(HC):
                                nc.tensor.matmul(ps[:, sl],
                                                 w[:, c, ob * 128:(ob + 1) * 128],
                                                 x_in[:, c, sl], start=(c == 0),
                                                 stop=(c == HC - 1))
                        if gc is not None:
                            th = sg.tile([128, R], F32, tag="th", bufs=2)
                            nc.scalar.activation(th[:], ps[:], act_fn, scale=scale,
                                                 bias=bias_cols[:, ob:ob + 1])
                            nc.vector.tensor_scalar(out=out_t[:, ob, :], in0=th[:],
                                                    scalar1=gc[:, ob:ob + 1],
                                                    scalar2=bec[:, ob:ob + 1],
                                                    op0=ALU.mult, op1=ALU.add)
                        else:
                            nc.scalar.activation(out_t[:, ob, :], ps[:], act_fn,
                                                 scale=scale,
                                                 bias=bias_cols[:, ob:ob + 1])
                    return out_t

                zpT = {}
                for side, x_d in (("b", zb_d), ("a", za_d)):
                    xT = load_xT(side, x_d)
                    h1 = layer(xT, w1, HC, AF.Tanh, a1_bc, ab1, g1c, be1c, "big", F32R)
                    h2 = layer(h1, w2, HC, AF.Tanh, a2_bc, ab2, g2c, be2c, "big", F32R)
                    zpT[side] = layer(h2, w3, DC, AF.Silu, 1.0, b3c, None, None,
                                      f"zp{side}", F32)

                    sq = sg.tile([128, DC, R], F32R, tag="sq")
                    nc.vector.tensor_mul(sq[:], zpT[side][:], zpT[side][:])
                    lnr = sg.tile([1, R], F32, tag="lnr")
                    for ns in range(NS):
                        sl = slice(ns * 512, (ns + 1) * 512)
                        ns2 = npn.tile([1, 512], F32, tag="smallps", name="ns2")
                        for c in range(DC):
                            nc.tensor.matmul(ns2[:], ones_k[:], sq[:, c, sl],
                                             start=(c == 0), stop=(c == DC - 1))
                        nc.scalar.activation(lnr[:, sl], ns2[:], AF.Ln)
                    if side == "b":
                        # zb_hat = zb_pT * (invnb/tau)[r]; gather it (the only gather)
                        invnt = sg.tile([1, R], F32, tag="invnt")
                        nc.scalar.activation(invnt[:], lnr[:], AF.Exp, scale=-0.5,
                                             bias=ln2_t)
                        bc = sg.tile([128, R], F32, tag="bc")
                        nc.gpsimd.partition_broadcast(bc[:], invnt[:])
                        hat = sg.tile([128, DC, R], BF16, tag="hat")
                        for c in range(DC):
                            nc.vector.tensor_mul(hat[:, c, :], zpT[side][:, c, :], bc[:])
                        nc.sync.dma_start(out=ag_in[:].rearrange("c p r -> p c r"),
                                          in_=hat[:])
                        nc.gpsimd.collective_compute(
                            "AllGather", ALU.bypass, replica_groups=[list(range(NC))],
                            ins=[ag_in[:].opt()], outs=[ag_out[:].opt()])
                        # invnb cols (for the pos_sim diag path): invnb = invnt/2
                        dnormb = dram.tile([1, R], F32, tag="dnormb")
                        nc.sync.dma_start(out=dnormb[:], in_=invnt[:])
                        ivb = sg.tile([128, NB], F32, tag="ivb")
                        nc.sync.dma_start(
                            out=ivb[:],
                            in_=dnormb[0:1, :].rearrange("o (b p) -> (o p) b", b=NB))
                        nc.vector.tensor_scalar(out=invnb_cols[:], in0=ivb[:],
                                                scalar1=float(TAU), scalar2=None,
                                                op0=ALU.mult)
                    else:
                        # za stays raw; invna applied at exp time (per-partition scale)
                        invn = sg.tile([1, R], F32, tag="invnt")
                        nc.scalar.activation(invn[:], lnr[:], AF.Exp, scale=-0.5)
                        dnorma = dram.tile([1, R], F32, tag="dnorma")
                        nc.sync.dma_start(out=dnorma[:], in_=invn[:])
                        nc.sync.dma_start(
                            out=invna_cols[:],
                            in_=dnorma[0:1, :].rearrange("o (b p) -> (o p) b", b=NB))

                nc.vector.tensor_copy(za_r[:], zpT["a"][:])

                # output slab: transpose zpT back to natural and store
                for rb in range(NB):
                    tp4 = tpp.tile([128, 2 * D], F32, tag="tp")
                    for k, side in enumerate(("a", "b")):
                        for ob in range(DC):
                            nc.tensor.transpose(
                                tp4[:, k * D + ob * 128:k * D + (ob + 1) * 128],
                                zpT[side][:, ob, rb * 128:(rb + 1) * 128], ident[:])
                    ost = sg.tile([128, 2 * D], F32, tag="ost", bufs=2)
                    nc.vector.tensor_copy(ost[:], tp4[:])
                    nc.sync.dma_start(out=out_d[rb * 128:(rb + 1) * 128, :], in_=ost[:])

                # pos_sim (diag of m): exp(rawdot*invna*invnb/tau); psg partial
                prod = sg.tile([128, DC, R], F32R, tag="sq")
                nc.vector.tensor_mul(prod[:], zpT["a"][:], zpT["b"][:])
                rd_ps = npn.tile([128, NB, 2], F32, tag="smallps", name="rd_ps")
                for ib in range(NB):
                    for c in range(DC):
                        nc.tensor.matmul(rd_ps[:, ib, :],
                                         prod[:, c, ib * 128:(ib + 1) * 128],
                                         ones2[:], start=(c == 0), stop=(c == DC - 1))
                t1 = sg.tile([128, NB], F32, tag="t1")
                nc.vector.tensor_mul(t1[:], rd_ps[:, :, 0], invna_cols[:])
                t2 = sg.tile([128, NB], F32, tag="t2")
                nc.vector.tensor_mul(t2[:], t1[:], invnb_cols[:])
                ps_cols = sg.tile([128, NB], F32, tag="t3")
                nc.scalar.activation(ps_cols[:], t2[:], AF.Exp, scale=float(1.0 / TAU))
                zcols = sg.tile([128, NB], F32, tag="zcols")
                nc.vector.memset(zcols[:], 0.0)
                nc.vector.tensor_copy(ps_cols_r[:, :, 1], zcols[:])
                nc.vector.tensor_copy(ps_cols_r[:, :, 0], ps_cols[:])
                psg_ps = npn.tile([G, 2], F32, tag="smallps", name="psg_ps")
                for ib in range(NB):
                    nc.tensor.matmul(psg_ps[:], hot_r[:, ib, 0:G], ps_cols_r[:, ib, :],
                                     start=(ib == 0), stop=(ib == NB - 1))
                nc.vector.tensor_copy(psg_sb[:], psg_ps[:, 0:1])

            # ---------------- Phase C: similarity slab ----------------
            with tc.tile_pool(name="strm", bufs=3) as strm, \
                 tc.tile_pool(name="pospool", bufs=3) as pp, \
                 tc.tile_pool(name="mpool", bufs=3) as mpo, \
                 tc.tile_pool(name="scr", bufs=2) as scp, \
                 tc.tile_pool(name="sps", bufs=2, space="PSUM") as sps, \
                 tc.tile_pool(name="ups", bufs=2, space="PSUM") as ups, \
                 tc.tile_pool(name="tiny", bufs=4) as tiny:
                for jc in range(NB):
                    zb_sl = strm.tile([128, DC, R], BF16, tag="zb_sl", name="zb_sl")
                    nc.sync.dma_start(out=zb_sl[:],
                                      in_=ag_out[jc].rearrange("c p r -> p c r"))
                    u_ps = ups.tile([128, R], F32, tag="u", name="u_ps")
                    ms, poss, posTs = [], [], []
                    # pass 1: stream all S matmuls (dense PE) + exp
                    for ib in range(NB):
                        ibs = slice(ib * 128, (ib + 1) * 128)
                        acol = ib * NB + jc
                        pos_t = pp.tile([128, R], BF16, tag="pos", name="pos_t", bufs=9)
                        nc.gpsimd.dma_start(out=pos_t[:],
                                            in_=pos_d[ibs, jc * R:(jc + 1) * R])
                        posT_t = pp.tile([128, R], BF16, tag="posT", name="posT_t",
                                         bufs=9)
                        nc.gpsimd.dma_start(out=posT_t[:],
                                            in_=posT_d[ibs, jc * R:(jc + 1) * R])
                        sa = sps.tile([128, R], F32, tag="S", name="sa")
                        for ns in range(NS):
                            sl = slice(ns * 512, (ns + 1) * 512)
                            for c in range(DC):
                                nc.tensor.matmul(sa[:, sl], za_r[:, c, ibs],
                                                 zb_sl[:, c, sl],
                                                 start=(c == 0), stop=(c == DC - 1))
                        m_a = mpo.tile([128, R], BF16, tag="ma", name="m_a", bufs=9)
                        nc.scalar.activation(m_a[:], sa[:], AF.Exp,
                                             scale=invna_cols[:, ib:ib + 1],
                                             accum_out=rsacc[:, acol:acol + 1])
                        ms.append(m_a); poss.append(pos_t); posTs.append(posT_t)
                    # pass 2: dot products + group/colsum/dot_b matmuls
                    for ib in range(NB):
                        acol = ib * NB + jc
                        m_a, pos_t, posT_t = ms[ib], poss[ib], posTs[ib]
                        scr_a = scp.tile([128, R], BF16, tag="scra", name="scr_a")
                        nc.vector.scalar_tensor_tensor(
                            out=scr_a[:], in0=m_a[:], scalar=1.0, in1=pos_t[:],
                            op0=ALU.mult, op1=ALU.mult,
                            accum_out=daacc[:, acol:acol + 1])
                        prod_b = scp.tile([128, R], BF16, tag="prodb", name="prod_b")
                        nc.vector.tensor_mul(prod_b[:], m_a[:], posT_t[:])
                        for ns in range(NS):
                            sl = slice(ns * 512, (ns + 1) * 512)
                            nc.tensor.matmul(u_ps[0:65, sl], hot_r[:, ib, :], m_a[:, sl],
                                             start=(ib == 0), stop=(ib == NB - 1),
                                             skip_group_check=True)
                            nc.tensor.matmul(u_ps[96:97, sl], ones_kb[:], prod_b[:, sl],
                                             start=(ib == 0), stop=(ib == NB - 1),
                                             skip_group_check=True,
                                             tile_position=(0, 96))
                    csdb_st = scp.tile([128, R], F32, tag="csdb", name="csdb_st",
                                       bufs=2)
                    nc.scalar.copy(csdb_st[64:65, :], u_ps[64:65, :])
                    nc.scalar.copy(csdb_st[96:97, :], u_ps[96:97, :])
                    if jc < NB - 1:
                        nc.sync.dma_start(out=cca_in[0, jc].rearrange("(o f) -> o f", o=1),
                                          in_=csdb_st[64:65, :])
                        nc.sync.dma_start(out=cca_in[1, jc].rearrange("(o f) -> o f", o=1),
                                          in_=csdb_st[96:97, :])
                    else:
                        nc.sync.dma_start(
                            out=cc2_in[o3:o4].rearrange("(o f) -> o f", o=1),
                            in_=csdb_st[64:65, :])
                        nc.sync.dma_start(
                            out=cc2_in[o4:].rearrange("(o f) -> o f", o=1),
                            in_=csdb_st[96:97, :])
                    if jc == NB - 2:
                        nc.gpsimd.collective_compute(
                            "AllReduce", ALU.add, replica_groups=[list(range(NC))],
                            ins=[cca_in[:].opt()], outs=[cca_out[:].opt()])
                    for (g, lo, hi, first) in segs_by_jc[jc]:
                        if first:
                            nc.vector.reduce_sum(bs_sb[:, g:g + 1], u_ps[0:G, lo:hi],
                                                 axis=X)
                        else:
                            tmp = tiny.tile([G, 1], F32, tag="segtmp", name="segtmp")
                            nc.vector.reduce_sum(tmp[:], u_ps[0:G, lo:hi], axis=X)
                            nc.vector.tensor_add(bs_sb[:, g:g + 1], bs_sb[:, g:g + 1],
                                                 tmp[:])

            # ---------------- Phase D: local log-sums + allreduce ----------------
            with tc.tile_pool(name="ep", bufs=1) as ep, \
                 tc.tile_pool(name="eps", bufs=2, space="PSUM") as epp:
                red = ep.tile([128, 2, NB], F32)
                for k, acc in enumerate((daacc, rsacc)):
                    nc.vector.reduce_sum(red[:, k, :],
                                         acc[:].rearrange("p (ib jc) -> p ib jc", ib=NB),
                                         axis=X)
                nc.vector.tensor_scalar(out=red[:, 1, :], in0=red[:, 1, :],
                                        scalar1=EPS_G, scalar2=None, op0=ALU.add)
                la2 = ep.tile([128, 2], F32)
                lnscr = ep.tile([128, NB], F32)
                for k in range(2):
                    nc.scalar.activation(lnscr[:], red[:, k, :], AF.Ln,
                                         accum_out=la2[:, k:k + 1])
                la2r = ep.tile([128, 2], F32R)
                nc.vector.tensor_copy(la2r[:], la2[:])
                v01_ps = epp.tile([1, 2], F32)
                nc.tensor.matmul(v01_ps[:], ones_k[:], la2r[:], start=True, stop=True)
                nc.vector.tensor_copy(v01_sb[:], v01_ps[:])

                nc.sync.dma_start(out=cc2_in[0:o1].rearrange("(g h) -> g h", g=G),
                                  in_=bs_sb[:])
                nc.sync.dma_start(out=cc2_in[o1:o2].rearrange("(g o) -> g o", g=G),
                                  in_=psg_sb[:])
                nc.sync.dma_start(out=cc2_in[o2:o3].rearrange("(o f) -> o f", o=1),
                                  in_=v01_sb[:])
                nc.gpsimd.collective_compute(
                    "AllReduce", ALU.add, replica_groups=[list(range(NC))],
                    ins=[cc2_in[:].opt()], outs=[cc2_out[:].opt()])

                # ---------------- Phase E: final scalar loss ----------------
                bs_f = ep.tile([G, G], F32)
                nc.sync.dma_start(out=bs_f[:],
                                  in_=cc2_out[0:o1].rearrange("(g h) -> g h", g=G))
                psg_f = ep.tile([G, 1], F32)
                nc.sync.dma_start(out=psg_f[:],
                                  in_=cc2_out[o1:o2].rearrange("(g o) -> g o", g=G))
                # full colsum/dot_b rows -> [128, 64] col layout (jc<7 from the
                # early allreduce, jc=7 from the final one)
                csdb = ep.tile([128, 2, G], F32)
                for k in range(2):
                    nc.sync.dma_start(
                        out=csdb[:, k, 0:(NB - 1) * NB],
                        in_=cca_out[k].rearrange("j (b p) -> p (j b)", p=128))
                src7 = cc2_out[o3:o4] if True else None
                nc.sync.dma_start(
                    out=csdb[:, 0, (NB - 1) * NB:],
                    in_=cc2_out[o3:o4].rearrange("(b p) -> p b", p=128))
                nc.sync.dma_start(
                    out=csdb[:, 1, (NB - 1) * NB:],
                    in_=cc2_out[o4:].rearrange("(b p) -> p b", p=128))
                # v2 = sum ln(dot_b), v3 = sum ln(colsum+eps) over ALL rows (identical
                # on every core -> bypasses the allreduce)
                nc.vector.tensor_scalar(out=csdb[:, 0, :], in0=csdb[:, 0, :],
                                        scalar1=EPS_G, scalar2=None, op0=ALU.add)
                lb2 = ep.tile([128, 2], F32)
                lnscr2 = ep.tile([128, G], F32)
                nc.scalar.activation(lnscr2[:], csdb[:, 1, :], AF.Ln,
                                     accum_out=lb2[:, 0:1])
                nc.scalar.activation(lnscr2[:], csdb[:, 0, :], AF.Ln,
                                     accum_out=lb2[:, 1:2])
                lb2r = ep.tile([128, 2], F32R)
                nc.vector.tensor_copy(lb2r[:], lb2[:])
                v23_ps = epp.tile([1, 2], F32)
                nc.tensor.matmul(v23_ps[:], ones_k[:], lb2r[:], start=True, stop=True)

                L4 = ep.tile([G, 4], F32)
                nc.sync.dma_start(out=L4[:, 0:1],
                                  in_=cc2_out[o1:o2].rearrange("(g o) -> g o", g=G))
                gs = ep.tile([G, 1], F32)
                eyescr = ep.tile([G, G], F32)
                nc.vector.scalar_tensor_tensor(out=eyescr[:], in0=bs_f[:], scalar=1.0,
                                               in1=eye64[:], op0=ALU.mult, op1=ALU.mult,
                                               accum_out=gs[:])
                neg1r = ep.tile([G, 1], F32)
                nc.vector.reduce_sum(neg1r[:], bs_f[:], axis=X)
                nc.vector.scalar_tensor_tensor(out=L4[:, 2:3], in0=neg1r[:],
                                               scalar=EPS_L, in1=gs[:], op0=ALU.add,
                                               op1=ALU.subtract)
                bs_fr = ep.tile([G, G], F32R)
                nc.vector.tensor_copy(bs_fr[:], bs_f[:])
                neg0_ps = epp.tile([G, 2], F32)
                nc.tensor.matmul(neg0_ps[:], bs_fr[:], ones2[0:G, :], start=True,
                                 stop=True)
                nc.vector.scalar_tensor_tensor(out=L4[:, 1:2], in0=neg0_ps[:, 0:1],
                                               scalar=EPS_L, in1=gs[:], op0=ALU.add,
                                               op1=ALU.subtract)
                nc.vector.scalar_tensor_tensor(out=L4[:, 3:4], in0=gs[:], scalar=EPS_L,
                                               in1=psg_f[:], op0=ALU.add,
                                               op1=ALU.subtract)
                L4ln = ep.tile([G, 4], F32)
                nc.scalar.activation(L4ln[:], L4[:], AF.Ln)
                L4r = ep.tile([G, 4], F32R)
                nc.vector.tensor_copy(L4r[:], L4ln[:])
                s4_ps = epp.tile([1, 4], F32)
                nc.tensor.matmul(s4_ps[:], ones_k[0:G, :], L4r[:], start=True, stop=True)

                vrow = ep.tile([1, 8], F32)
                nc.sync.dma_start(out=vrow[:, 0:2],
                                  in_=cc2_out[o2:o3].rearrange("(o f) -> o f", o=1))
                nc.vector.tensor_copy(vrow[:, 2:4], v23_ps[:])
                nc.vector.tensor_copy(vrow[:, 4:8], s4_ps[:])
                vscr = ep.tile([1, 8], F32)
                loss_sb = ep.tile([1, 1], F32)
                nc.vector.scalar_tensor_tensor(out=vscr[:], in0=vrow[:], scalar=1.0,
                                               in1=coef_sb[:], op0=ALU.mult, op1=ALU.mult,
                                               accum_out=loss_sb[:])
                nc.sync.dma_start(out=loss_d[:], in_=loss_sb[:])

    nc.compile()
    return nc


def kernel(**inputs):
    global LAST_RESULTS
    from concourse.bass_utils import run_bass_kernel_spmd
    import ml_dtypes

    batch = np.asarray(inputs["batch"], dtype=np.int64)
    key = batch.tobytes()
    if _PROGRAM_CACHE.get("key") != key:
        _PROGRAM_CACHE["prog"] = _build_program(batch)
        _PROGRAM_CACHE["key"] = key
    nc = _PROGRAM_CACHE["prog"]

    za = np.asarray(inputs["za"], dtype=np.float32)
    zb = np.asarray(inputs["zb"], dtype=np.float32)
    pos = np.asarray(inputs["pos"], dtype=np.float32)
    bf16 = ml_dtypes.bfloat16
    pos_bf = pos.astype(bf16)
    posT_bf = np.ascontiguousarray(pos.T).astype(bf16)
    hot = np.zeros((N, 65), dtype=bf16)
    hot[np.arange(N), batch] = 1
    hot[:, 64] = 1

    def cols(v, nb):
        return np.asarray(v, dtype=np.float32).reshape(nb, 128).T

    prm = np.zeros((128, 32), dtype=np.float32)
    prm[:, 0:4] = cols(inputs["b1"], 4)
    prm[:, 4:8] = cols(inputs["g1"], 4)
    prm[:, 8:12] = cols(inputs["be1"], 4)
    prm[:, 12:16] = cols(inputs["b2"], 4)
    prm[:, 16:20] = cols(inputs["g2"], 4)
    prm[:, 20:24] = cols(inputs["be2"], 4)
    prm[:, 24:26] = cols(inputs["b3"], 2)
    prm[:, 26] = np.float32(np.asarray(inputs["a1"]).reshape(-1)[0])
    prm[:, 27] = np.float32(np.asarray(inputs["a2"]).reshape(-1)[0])
    prm[:, 28] = np.float32(np.log(1.0 / TAU))

    coef = np.array([[-LAM / N, LAM / N, -(1.0 - LAM) / N, (1.0 - LAM) / N,
                      ALPHA / G - BETA / G, -ALPHA / (2 * G), -ALPHA / (2 * G),
                      BETA / G]], dtype=np.float32)

    shared = {
        "W1": np.asarray(inputs["W1"], dtype=np.float32),
        "W2": np.asarray(inputs["W2"], dtype=np.float32),
        "W3": np.asarray(inputs["W3"], dtype=np.float32),
        "prm": prm, "coef": coef,
    }
    in_maps = []
    for c in range(NC):
        sl = slice(c * R, (c + 1) * R)
        m = dict(shared)
        m["za_s"] = za[sl]
        m["zb_s"] = zb[sl]
        m["pos_s"] = pos_bf[sl]
        m["posT_s"] = posT_bf[sl]
        m["hot_s"] = hot[sl]
        in_maps.append(m)

    res = run_bass_kernel_spmd(nc, in_maps, list(range(NC)))
    LAST_RESULTS = res
    out = np.concatenate([res.results[c]["out_s"] for c in range(NC)], axis=0)
    loss = np.float32(res.results[0]["loss"][0, 0])
    return loss, out


# revision 29
# speedup vs baseline: 1.1178x; 1.0201x over previous
"""Bass/Trainium2 kernel for nn_EnhancedContrast (8-core SPMD).

Sharding: rows (N=8192) split across 8 cores, 1024 rows each. Each core:
  - runs the projection MLP on its za/zb row-slab (activation-transposed layout,
    zb first so the single all-gather of normalized zb overlaps za's MLP),
  - computes its row-slab of m = exp(cos/tau): rowsum/dot_a reduce locally along
    the free dim; colsum partials ride a ones-column on the one-hot matmul and
    dot_b partials come from a host-transposed bf16 pos^T via a ones-matmul
    partition reduction,
  - one AllReduce (~84KB) combines batch_sim/pos_sim_graph/colsum/dot_b/log
    partials; every core then computes the identical scalar loss.
"""
import sys

sys.path.insert(0, "/opt/trn_rl_repo")

import numpy as np

N, H, D, G = 8192, 512, 256, 64
TAU, LAM, ALPHA, BETA = 0.5, 0.5, 1.0, 1.0
EPS_G, EPS_L = 1e-6, 1e-5
NC = 8            # cores
R = N // NC       # rows per core = 1024
NB = R // 128     # 128-row blocks per core = 8
HC = H // 128     # hidden chunks = 4
DC = D // 128     # proj-dim chunks = 2
NS = R // 512     # 512-wide moving slices per 1024 = 2

LAST_RESULTS = None  # stashed BassKernelResults for test.py
_PROGRAM_CACHE = {}


def _build_program(batch_np):
    import concourse.mybir as mybir
    import concourse.tile as tile
    from concourse import bacc
    from concourse.masks import make_identity

    F32 = mybir.dt.float32
    F32R = mybir.dt.float32r
    BF16 = mybir.dt.bfloat16
    I32 = mybir.dt.int32
    AF = mybir.ActivationFunctionType
    ALU = mybir.AluOpType
    X = mybir.AxisListType.X

    # group segments along the full column axis (batch is sorted)
    bounds = np.searchsorted(batch_np, np.arange(G + 1))
    segs_by_jc = [[] for _ in range(NB)]
    for g in range(G):
        lo, hi = int(bounds[g]), int(bounds[g + 1])
        first = True
        j = lo
        while j < hi:
            jc = j // R
            e = min(hi, (jc + 1) * R)
            segs_by_jc[jc].append((g, j - jc * R, e - jc * R, first))
            first = False
            j = e

    nc = bacc.Bacc("TRN2", target_bir_lowering=False, debug=False, num_devices=NC)

    # ---- I/O ----
    za_d = nc.dram_tensor("za_s", [R, H], F32, kind="ExternalInput")
    zb_d = nc.dram_tensor("zb_s", [R, H], F32, kind="ExternalInput")
    pos_d = nc.dram_tensor("pos_s", [R, N], BF16, kind="ExternalInput")
    posT_d = nc.dram_tensor("posT_s", [R, N], BF16, kind="ExternalInput")
    hot_d = nc.dram_tensor("hot_s", [R, 65], BF16, kind="ExternalInput")
    w1_d = nc.dram_tensor("W1", [H, H], F32R, kind="ExternalInput")
    w2_d = nc.dram_tensor("W2", [H, H], F32R, kind="ExternalInput")
    w3_d = nc.dram_tensor("W3", [H, D], F32R, kind="ExternalInput")
    prm_d = nc.dram_tensor("prm", [128, 32], F32, kind="ExternalInput")
    coef_d = nc.dram_tensor("coef", [1, 8], F32, kind="ExternalInput")
    out_d = nc.dram_tensor("out_s", [R, 2 * D], F32, kind="ExternalOutput")
    loss_d = nc.dram_tensor("loss", [1, 1], F32, kind="ExternalOutput")

    with tile.TileContext(nc) as tc:
        with tc.tile_pool(name="consts", bufs=1) as consts, \
             tc.tile_pool(name="live", bufs=1) as live, \
             tc.tile_pool(name="dram", bufs=1, space="DRAM") as dram:
            # packed per-partition params: b1c g1c be1c b2c g2c be2c (4 cols each),
            # b3c (2), a1, a2, ln2 -> 29 cols used
            prm = consts.tile([128, 32], F32)
            nc.sync.dma_start(out=prm[:], in_=prm_d[:])
            b1c, g1c, be1c = prm[:, 0:4], prm[:, 4:8], prm[:, 8:12]
            b2c, g2c, be2c = prm[:, 12:16], prm[:, 16:20], prm[:, 20:24]
            b3c = prm[:, 24:26]
            a1_bc, a2_bc = prm[:, 26:27], prm[:, 27:28]
            ln2_t = prm[0:1, 28:29]
            coef_sb = consts.tile([1, 8], F32)
            nc.sync.dma_start(out=coef_sb[:], in_=coef_d[:])

            ident = consts.tile([128, 128], F32)
            make_identity(nc, ident[:])
            ones_k32 = consts.tile([128, 1], F32)
            nc.vector.memset(ones_k32[:], 1.0)
            ones_k = consts.tile([128, 1], F32R)
            nc.vector.tensor_copy(ones_k[:], ones_k32[:])
            ones_kb = consts.tile([128, 1], BF16)
            nc.vector.tensor_copy(ones_kb[:], ones_k32[:])
            ones2_32 = consts.tile([128, 2], F32)
            nc.vector.memset(ones2_32[:], 1.0)
            ones2 = consts.tile([128, 2], F32R)
            nc.vector.tensor_copy(ones2[:], ones2_32[:])

            ab1 = consts.tile([128, HC], F32)
            nc.vector.tensor_scalar(out=ab1[:], in0=b1c, scalar1=a1_bc, scalar2=None,
                                    op0=ALU.mult)
            ab2 = consts.tile([128, HC], F32)
            nc.vector.tensor_scalar(out=ab2[:], in0=b2c, scalar1=a2_bc, scalar2=None,
                                    op0=ALU.mult)

            # one-hot (64 groups + ones column) per row-block, host-prepared
            hot_r = consts.tile([128, NB, 65], BF16)
            nc.sync.dma_start(out=hot_r[:],
                              in_=hot_d[:].rearrange("(b p) g -> p b g", p=128))

            # eye64 for diag extraction in the epilogue
            iota_i = consts.tile([1, G], I32)
            nc.gpsimd.iota(iota_i[:], pattern=[[1, G]], base=0, channel_multiplier=0)
            iota_bc_i = consts.tile([128, G], I32)
            nc.gpsimd.partition_broadcast(iota_bc_i[:], iota_i[:])
            iota_f = consts.tile([128, G], F32)
            nc.vector.tensor_copy(iota_f[:], iota_bc_i[:])
            iota_col_i = consts.tile([128, 1], I32)
            nc.gpsimd.iota(iota_col_i[:], pattern=[[0, 1]], base=0, channel_multiplier=1)
            iota_col = consts.tile([128, 1], F32)
            nc.vector.tensor_copy(iota_col[:], iota_col_i[:])
            eye64 = consts.tile([G, G], F32)
            nc.vector.tensor_scalar(out=eye64[:], in0=iota_f[0:G, :],
                                    scalar1=iota_col[0:G, :], scalar2=None,
                                    op0=ALU.is_equal)

            # long-lived similarity-phase tensors
            za_r = live.tile([128, DC, R], BF16)     # stationary for the m-slab matmuls
            invna_cols = live.tile([128, NB], F32)
            invnb_cols = live.tile([128, NB], F32)
            rsacc = live.tile([128, NB * NB], F32)   # rowsum partials, col = ib*NB+jc
            daacc = live.tile([128, NB * NB], F32)
            bs_sb = live.tile([G, G], F32)
            nc.vector.memset(bs_sb[:], 0.0)
            psg_sb = live.tile([G, 1], F32)
            v01_sb = live.tile([1, 2], F32)
            ps_cols_r = live.tile([128, NB, 2], BF16)

            # collective buffers
            warm_in = dram.tile([16], F32)
            warm_out = dram.tile([NC, 16], F32, addr_space="Shared")
            ag_in = dram.tile([DC, 128, R], BF16)
            ag_out = dram.tile([NC, DC, 128, R], BF16, addr_space="Shared")
            # early allreduce: cs/db partial rows for jc 0..6 (overlaps the tail
            # of phase C); final allreduce: bs, psg, v01, cs/db for jc 7
            cca_in = dram.tile([2, NB - 1, R], F32)
            cca_out = dram.tile([2, NB - 1, R], F32, addr_space="Shared")
            o1, o2, o3, o4 = G * G, G * G + G, G * G + G + 2, G * G + G + 2 + R
            CC2 = o4 + R
            cc2_in = dram.tile([CC2], F32)
            cc2_out = dram.tile([CC2], F32, addr_space="Shared")

            wz = consts.tile([1, 16], F32)
            nc.vector.memset(wz[:], 0.0)
            nc.sync.dma_start(out=warm_in[:].rearrange("(o f) -> o f", o=1), in_=wz[:])
            nc.gpsimd.collective_compute(
                "AllGather", ALU.bypass, replica_groups=[list(range(NC))],
                ins=[warm_in[:].opt()], outs=[warm_out[:].opt()])

            # ------------- Phase A: MLP (zb first; its gather overlaps za) -------------
            with tc.tile_pool(name="wpool", bufs=1) as wp, \
                 tc.tile_pool(name="mlp", bufs=1) as mp, \
                 tc.tile_pool(name="mps", bufs=2, space="PSUM") as mpp, \
                 tc.tile_pool(name="tps", bufs=2, space="PSUM") as tpp, \
                 tc.tile_pool(name="stg", bufs=1) as sg, \
                 tc.tile_pool(name="npsn", bufs=2, space="PSUM") as npn:
                w1 = wp.tile([128, HC, H], F32R)
                nc.sync.dma_start(out=w1[:], in_=w1_d[:].rearrange("(c p) o -> p c o", p=128))
                w2 = wp.tile([128, HC, H], F32R)
                nc.sync.dma_start(out=w2[:], in_=w2_d[:].rearrange("(c p) o -> p c o", p=128))
                w3 = wp.tile([128, HC, D], F32R)
                nc.sync.dma_start(out=w3[:], in_=w3_d[:].rearrange("(c p) o -> p c o", p=128))

                def load_xT(side, x_d):
                    xT = mp.tile([128, HC, R], F32R, tag="big", bufs=4, name=f"xT{side}")
                    for rb in range(NB):
                        st = sg.tile([128, H], F32, tag="stage", bufs=3)
                        nc.sync.dma_start(out=st[:], in_=x_d[rb * 128:(rb + 1) * 128, :])
                        tp4 = tpp.tile([128, H], F32, tag="tp")
                        for c in range(HC):
                            nc.tensor.transpose(tp4[:, c * 128:(c + 1) * 128],
                                                st[:, c * 128:(c + 1) * 128], ident[:])
                        nc.vector.tensor_copy(
                            xT[:, :, rb * 128:(rb + 1) * 128],
                            tp4[:].rearrange("p (c q) -> p c q", c=HC))
                    return xT

                def layer(x_in, w, n_ob, act_fn, scale, bias_cols, gc, bec, out_tag,
                          out_dtype):
                    bufs = 4 if out_tag == "big" else 1
                    out_t = mp.tile([128, n_ob, R], out_dtype, tag=out_tag, bufs=bufs,
                                    name=f"L{out_tag}{n_ob}")
                    for ob in range(n_ob):
                        ps = mpp.tile([128, R], F32, tag="mm")
                        for c in range(HC):
                            for ns in range(NS):
                                sl = slice(ns * 512, (ns + 1) * 512)
                                nc.tensor.matmul(ps[:, sl],
                                                 w[:, c, ob * 128:(ob + 1) * 128],
                                                 x_in[:, c, sl], start=(c == 0),
                                                 stop=(c == HC - 1),
                                                 skip_group_check=True)
                        if gc is not None:
                            th = sg.tile([128, R], F32, tag="th", bufs=2)
                            nc.scalar.activation(th[:], ps[:], act_fn, scale=scale,
                                                 bias=bias_cols[:, ob:ob + 1])
                            nc.vector.tensor_scalar(out=out_t[:, ob, :], in0=th[:],
                                                    scalar1=gc[:, ob:ob + 1],
                                                    scalar2=bec[:, ob:ob + 1],
                                                    op0=ALU.mult, op1=ALU.add)
                        else:
                            nc.scalar.activation(out_t[:, ob, :], ps[:], act_fn,
                                                 scale=scale,
                                                 bias=bias_cols[:, ob:ob + 1])
                    return out_t

                zpT = {}
                for side, x_d in (("b", zb_d), ("a", za_d)):
                    xT = load_xT(side, x_d)
                    h1 = layer(xT, w1, HC, AF.Tanh, a1_bc, ab1, g1c, be1c, "big", F32R)
                    h2 = layer(h1, w2, HC, AF.Tanh, a2_bc, ab2, g2c, be2c, "big", F32R)
                    zpT[side] = layer(h2, w3, DC, AF.Silu, 1.0, b3c, None, None,
                                      f"zp{side}", F32)

                    sq = sg.tile([128, DC, R], F32R, tag="sq")
                    nc.vector.tensor_mul(sq[:], zpT[side][:], zpT[side][:])
                    lnr = sg.tile([1, R], F32, tag="lnr")
                    for ns in range(NS):
                        sl = slice(ns * 512, (ns + 1) * 512)
                        ns2 = npn.tile([1, 512], F32, tag="smallps", name="ns2")
                        for c in range(DC):
                            nc.tensor.matmul(ns2[:], ones_k[:], sq[:, c, sl],
                                             start=(c == 0), stop=(c == DC - 1))
                        nc.scalar.activation(lnr[:, sl], ns2[:], AF.Ln)
                    if side == "b":
                        # zb_hat = zb_pT * (invnb/tau)[r]; gather it (the only gather)
                        invnt = sg.tile([1, R], F32, tag="invnt")
                        nc.scalar.activation(invnt[:], lnr[:], AF.Exp, scale=-0.5,
                                             bias=ln2_t)
                        bc = sg.tile([128, R], F32, tag="bc")
                        nc.gpsimd.partition_broadcast(bc[:], invnt[:])
                        hat = sg.tile([128, DC, R], BF16, tag="hat")
                        for c in range(DC):
                            nc.vector.tensor_mul(hat[:, c, :], zpT[side][:, c, :], bc[:])
                        nc.sync.dma_start(out=ag_in[:].rearrange("c p r -> p c r"),
                                          in_=hat[:])
                        nc.gpsimd.collective_compute(
                            "AllGather", ALU.bypass, replica_groups=[list(range(NC))],
                            ins=[ag_in[:].opt()], outs=[ag_out[:].opt()])
                        # invnb cols (for the pos_sim diag path): invnb = invnt/2
                        dnormb = dram.tile([1, R], F32, tag="dnormb")
                        nc.sync.dma_start(out=dnormb[:], in_=invnt[:])
                        ivb = sg.tile([128, NB], F32, tag="ivb")
                        nc.sync.dma_start(
                            out=ivb[:],
                            in_=dnormb[0:1, :].rearrange("o (b p) -> (o p) b", b=NB))
                        nc.vector.tensor_scalar(out=invnb_cols[:], in0=ivb[:],
                                                scalar1=float(TAU), scalar2=None,
                                                op0=ALU.mult)
                    else:
                        # za stays raw; invna applied at exp time (per-partition scale)
                        invn = sg.tile([1, R], F32, tag="invnt")
                        nc.scalar.activation(invn[:], lnr[:], AF.Exp, scale=-0.5)
                        dnorma = dram.tile([1, R], F32, tag="dnorma")
                        nc.sync.dma_start(out=dnorma[:], in_=invn[:])
                        nc.sync.dma_start(
                            out=invna_cols[:],
                            in_=dnorma[0:1, :].rearrange("o (b p) -> (o p) b", b=NB))

                nc.vector.tensor_copy(za_r[:], zpT["a"][:])

                # output slab: transpose zpT back to natural and store
                for rb in range(NB):
                    tp4 = tpp.tile([128, 2 * D], F32, tag="tp")
                    for k, side in enumerate(("a", "b")):
                        for ob in range(DC):
                            nc.tensor.transpose(
                                tp4[:, k * D + ob * 128:k * D + (ob + 1) * 128],
                                zpT[side][:, ob, rb * 128:(rb + 1) * 128], ident[:])
                    ost = sg.tile([128, 2 * D], F32, tag="ost", bufs=2)
                    nc.vector.tensor_copy(ost[:], tp4[:])
                    nc.sync.dma_start(out=out_d[rb * 128:(rb + 1) * 128, :], in_=ost[:])

                # pos_sim (diag of m): exp(rawdot*invna*invnb/tau); psg partial
                prod = sg.tile([128, DC, R], F32R, tag="sq")
                nc.vector.tensor_mul(prod[:], zpT["a"][:], zpT["b"][:])
                rd_ps = npn.tile([128, NB, 2], F32, tag="smallps", name="rd_ps")
                for ib in range(NB):
                    for c in range(DC):
                        nc.tensor.matmul(rd_ps[:, ib, :],
                                         prod[:, c, ib * 128:(ib + 1) * 128],
                                         ones2[:], start=(c == 0), stop=(c == DC - 1))
                t1 = sg.tile([128, NB], F32, tag="t1")
                nc.vector.tensor_mul(t1[:], rd_ps[:, :, 0], invna_cols[:])
                t2 = sg.tile([128, NB], F32, tag="t2")
                nc.vector.tensor_mul(t2[:], t1[:], invnb_cols[:])
                ps_cols = sg.tile([128, NB], F32, tag="t3")
                nc.scalar.activation(ps_cols[:], t2[:], AF.Exp, scale=float(1.0 / TAU))
                zcols = sg.tile([128, NB], F32, tag="zcols")
                nc.vector.memset(zcols[:], 0.0)
                nc.vector.tensor_copy(ps_cols_r[:, :, 1], zcols[:])
                nc.vector.tensor_copy(ps_cols_r[:, :, 0], ps_cols[:])
                psg_ps = npn.tile([G, 2], F32, tag="smallps", name="psg_ps")
                for ib in range(NB):
                    nc.tensor.matmul(psg_ps[:], hot_r[:, ib, 0:G], ps_cols_r[:, ib, :],
                                     start=(ib == 0), stop=(ib == NB - 1))
                nc.vector.tensor_copy(psg_sb[:], psg_ps[:, 0:1])

            # ---------------- Phase C: similarity slab ----------------
            with tc.tile_pool(name="strm", bufs=3) as strm, \
                 tc.tile_pool(name="pospool", bufs=3) as pp, \
                 tc.tile_pool(name="mpool", bufs=3) as mpo, \
                 tc.tile_pool(name="scr", bufs=2) as scp, \
                 tc.tile_pool(name="sps", bufs=2, space="PSUM") as sps, \
                 tc.tile_pool(name="ups", bufs=2, space="PSUM") as ups, \
                 tc.tile_pool(name="tiny", bufs=4) as tiny:
                for jc in range(NB):
                    zb_sl = strm.tile([128, DC, R], BF16, tag="zb_sl", name="zb_sl")
                    nc.sync.dma_start(out=zb_sl[:],
                                      in_=ag_out[jc].rearrange("c p r -> p c r"))
                    u_ps = ups.tile([128, R], F32, tag="u", name="u_ps")
                    ms, poss, posTs = [], [], []
                    # pass 1: stream all S matmuls (dense PE) + exp
                    for ib in range(NB):
                        ibs = slice(ib * 128, (ib + 1) * 128)
                        acol = ib * NB + jc
                        pos_t = pp.tile([128, R], BF16, tag="pos", name="pos_t", bufs=9)
                        nc.gpsimd.dma_start(out=pos_t[:],
                                            in_=pos_d[ibs, jc * R:(jc + 1) * R])
                        posT_t = pp.tile([128, R], BF16, tag="posT", name="posT_t",
                                         bufs=9)
                        nc.gpsimd.dma_start(out=posT_t[:],
                                            in_=posT_d[ibs, jc * R:(jc + 1) * R])
                        sa = sps.tile([128, R], F32, tag="S", name="sa")
                        for c in range(DC):
                            for ns in range(NS):
                                sl = slice(ns * 512, (ns + 1) * 512)
                                nc.tensor.matmul(sa[:, sl], za_r[:, c, ibs],
                                                 zb_sl[:, c, sl],
                                                 start=(c == 0), stop=(c == DC - 1),
                                                 skip_group_check=True)
                        m_a = mpo.tile([128, R], BF16, tag="ma", name="m_a", bufs=9)
                        nc.scalar.activation(m_a[:], sa[:], AF.Exp,
                                             scale=invna_cols[:, ib:ib + 1],
                                             accum_out=rsacc[:, acol:acol + 1])
                        ms.append(m_a); poss.append(pos_t); posTs.append(posT_t)
                    # pass 2: dot products + group/colsum/dot_b matmuls
                    for ib in range(NB):
                        acol = ib * NB + jc
                        m_a, pos_t, posT_t = ms[ib], poss[ib], posTs[ib]
                        scr_a = scp.tile([128, R], BF16, tag="scra", name="scr_a")
                        nc.vector.scalar_tensor_tensor(
                            out=scr_a[:], in0=m_a[:], scalar=1.0, in1=pos_t[:],
                            op0=ALU.mult, op1=ALU.mult,
                            accum_out=daacc[:, acol:acol + 1])
                        prod_b = scp.tile([128, R], BF16, tag="prodb", name="prod_b")
                        nc.vector.tensor_mul(prod_b[:], m_a[:], posT_t[:])
                        for ns in range(NS):
                            sl = slice(ns * 512, (ns + 1) * 512)
                            nc.tensor.matmul(u_ps[0:65, sl], hot_r[:, ib, :], m_a[:, sl],
                                             start=(ib == 0), stop=(ib == NB - 1),
                                             skip_group_check=True)
                            nc.tensor.matmul(u_ps[96:97, sl], ones_kb[:], prod_b[:, sl],
                                             start=(ib == 0), stop=(ib == NB - 1),
                                             skip_group_check=True,
                                             tile_position=(0, 96))
                    csdb_st = scp.tile([128, R], F32, tag="csdb", name="csdb_st",
                                       bufs=2)
                    nc.scalar.copy(csdb_st[64:65, :], u_ps[64:65, :])
                    nc.scalar.copy(csdb_st[96:97, :], u_ps[96:97, :])
                    if jc < NB - 1:
                        nc.sync.dma_start(out=cca_in[0, jc].rearrange("(o f) -> o f", o=1),
                                          in_=csdb_st[64:65, :])
                        nc.sync.dma_start(out=cca_in[1, jc].rearrange("(o f) -> o f", o=1),
                                          in_=csdb_st[96:97, :])
                    else:
                        nc.sync.dma_start(
                            out=cc2_in[o3:o4].rearrange("(o f) -> o f", o=1),
                            in_=csdb_st[64:65, :])
                        nc.sync.dma_start(
                            out=cc2_in[o4:].rearrange("(o f) -> o f", o=1),
                            in_=csdb_st[96:97, :])
                    if jc == NB - 2:
                        nc.gpsimd.collective_compute(
                            "AllReduce", ALU.add, replica_groups=[list(range(NC))],
                            ins=[cca_in[:].opt()], outs=[cca_out[:].opt()])
                    for (g, lo, hi, first) in segs_by_jc[jc]:
                        if first:
                            nc.vector.reduce_sum(bs_sb[:, g:g + 1], u_ps[0:G, lo:hi],
                                                 axis=X)
                        else:
                            tmp = tiny.tile([G, 1], F32, tag="segtmp", name="segtmp")
                            nc.vector.reduce_sum(tmp[:], u_ps[0:G, lo:hi], axis=X)
                            nc.vector.tensor_add(bs_sb[:, g:g + 1], bs_sb[:, g:g + 1],
                                                 tmp[:])

            # ---------------- Phase D: local log-sums + allreduce ----------------
            with tc.tile_pool(name="ep", bufs=1) as ep, \
                 tc.tile_pool(name="eps", bufs=2, space="PSUM") as epp:
                red = ep.tile([128, 2, NB], F32)
                for k, acc in enumerate((daacc, rsacc)):
                    nc.vector.reduce_sum(red[:, k, :],
                                         acc[:].rearrange("p (ib jc) -> p ib jc", ib=NB),
                                         axis=X)
                nc.vector.tensor_scalar(out=red[:, 1, :], in0=red[:, 1, :],
                                        scalar1=EPS_G, scalar2=None, op0=ALU.add)
                la2 = ep.tile([128, 2], F32)
                lnscr = ep.tile([128, NB], F32)
                for k in range(2):
                    nc.scalar.activation(lnscr[:], red[:, k, :], AF.Ln,
                                         accum_out=la2[:, k:k + 1])
                la2r = ep.tile([128, 2], F32R)
                nc.vector.tensor_copy(la2r[:], la2[:])
                v01_ps = epp.tile([1, 2], F32)
                nc.tensor.matmul(v01_ps[:], ones_k[:], la2r[:], start=True, stop=True)
                nc.vector.tensor_copy(v01_sb[:], v01_ps[:])

                nc.sync.dma_start(out=cc2_in[0:o1].rearrange("(g h) -> g h", g=G),
                                  in_=bs_sb[:])
                nc.sync.dma_start(out=cc2_in[o1:o2].rearrange("(g o) -> g o", g=G),
                                  in_=psg_sb[:])
                nc.sync.dma_start(out=cc2_in[o2:o3].rearrange("(o f) -> o f", o=1),
                                  in_=v01_sb[:])
                nc.gpsimd.collective_compute(
                    "AllReduce", ALU.add, replica_groups=[list(range(NC))],
                    ins=[cc2_in[:].opt()], outs=[cc2_out[:].opt()])

                # ---------------- Phase E: final scalar loss ----------------
                bs_f = ep.tile([G, G], F32)
                nc.sync.dma_start(out=bs_f[:],
                                  in_=cc2_out[0:o1].rearrange("(g h) -> g h", g=G))
                psg_f = ep.tile([G, 1], F32)
                nc.sync.dma_start(out=psg_f[:],
                                  in_=cc2_out[o1:o2].rearrange("(g o) -> g o", g=G))
                # full colsum/dot_b rows -> [128, 64] col layout (jc<7 from the
                # early allreduce, jc=7 from the final one)
                csdb = ep.tile([128, 2, G], F32)
                for k in range(2):
                    nc.sync.dma_start(
                        out=csdb[:, k, 0:(NB - 1) * NB],
                        in_=cca_out[k].rearrange("j (b p) -> p (j b)", p=128))
                src7 = cc2_out[o3:o4] if True else None
                nc.sync.dma_start(
                    out=csdb[:, 0, (NB - 1) * NB:],
                    in_=cc2_out[o3:o4].rearrange("(b p) -> p b", p=128))
                nc.sync.dma_start(
                    out=csdb[:, 1, (NB - 1) * NB:],
                    in_=cc2_out[o4:].rearrange("(b p) -> p b", p=128))
                # v2 = sum ln(dot_b), v3 = sum ln(colsum+eps) over ALL rows (identical
                # on every core -> bypasses the allreduce)
                nc.vector.tensor_scalar(out=csdb[:, 0, :], in0=csdb[:, 0, :],
                                        scalar1=EPS_G, scalar2=None, op0=ALU.add)
                lb2 = ep.tile([128, 2], F32)
                lnscr2 = ep.tile([128, G], F32)
                nc.scalar.activation(lnscr2[:], csdb[:, 1, :], AF.Ln,
                                     accum_out=lb2[:, 0:1])
                nc.scalar.activation(lnscr2[:], csdb[:, 0, :], AF.Ln,
                                     accum_out=lb2[:, 1:2])
                lb2r = ep.tile([128, 2], F32R)
                nc.vector.tensor_copy(lb2r[:], lb2[:])
                v23_ps = epp.tile([1, 2], F32)
                nc.tensor.matmul(v23_ps[:], ones_k[:], lb2r[:], start=True, stop=True)

                L4 = ep.tile([G, 4], F32)
                nc.sync.dma_start(out=L4[:, 0:1],
                                  in_=cc2_out[o1:o2].rearrange("(g o) -> g o", g=G))
                gs = ep.tile([G, 1], F32)
                eyescr = ep.tile([G, G], F32)
                nc.vector.scalar_tensor_tensor(out=eyescr[:], in0=bs_f[:], scalar=1.0,
                                               in1=eye64[:], op0=ALU.mult, op1=ALU.mult,
                                               accum_out=gs[:])
                neg1r = ep.tile([G, 1], F32)
                nc.vector.reduce_sum(neg1r[:], bs_f[:], axis=X)
                nc.vector.scalar_tensor_tensor(out=L4[:, 2:3], in0=neg1r[:],
                                               scalar=EPS_L, in1=gs[:], op0=ALU.add,
                                               op1=ALU.subtract)
                bs_fr = ep.tile([G, G], F32R)
                nc.vector.tensor_copy(bs_fr[:], bs_f[:])
                neg0_ps = epp.tile([G, 2], F32)
                nc.tensor.matmul(neg0_ps[:], bs_fr[:], ones2[0:G, :], start=True,
                                 stop=True)
                nc.vector.scalar_tensor_tensor(out=L4[:, 1:2], in0=neg0_ps[:, 0:1],
                                               scalar=EPS_L, in1=gs[:], op0=ALU.add,
                                               op1=ALU.subtract)
                nc.vector.scalar_tensor_tensor(out=L4[:, 3:4], in0=gs[:], scalar=EPS_L,
                                               in1=psg_f[:], op0=ALU.add,
                                               op1=ALU.subtract)
                L4ln = ep.tile([G, 4], F32)
                nc.scalar.activation(L4ln[:], L4[:], AF.Ln)
                L4r = ep.tile([G, 4], F32R)
                nc.vector.tensor_copy(L4r[:], L4ln[:])
                s4_ps = epp.tile([1, 4], F32)
                nc.tensor.matmul(s4_ps[:], ones_k[0:G, :], L4r[:], start=True, stop=True)

                vrow = ep.tile([1, 8], F32)
                nc.sync.dma_start(out=vrow[:, 0:2],
                                  in_=cc2_out[o2:o3].rearrange("(o f) -> o f", o=1))
                nc.vector.tensor_copy(vrow[:, 2:4], v23_ps[:])
                nc.vector.tensor_copy(vrow[:, 4:8], s4_ps[:])
                vscr = ep.tile([1, 8], F32)
                loss_sb = ep.tile([1, 1], F32)
                nc.vector.scalar_tensor_tensor(out=vscr[:], in0=vrow[:], scalar=1.0,
                                               in1=coef_sb[:], op0=ALU.mult, op1=ALU.mult,
                                               accum_out=loss_sb[:])
                nc.sync.dma_start(out=loss_d[:], in_=loss_sb[:])

    nc.compile()
    return nc


def kernel(**inputs):
    global LAST_RESULTS
    from concourse.bass_utils import run_bass_kernel_spmd
    import ml_dtypes

    batch = np.asarray(inputs["batch"], dtype=np.int64)
    key = batch.tobytes()
    if _PROGRAM_CACHE.get("key") != key:
        _PROGRAM_CACHE["prog"] = _build_program(batch)
        _PROGRAM_CACHE["key"] = key
    nc = _PROGRAM_CACHE["prog"]

    za = np.asarray(inputs["za"], dtype=np.float32)
    zb = np.asarray(inputs["zb"], dtype=np.float32)
    pos = np.asarray(inputs["pos"], dtype=np.float32)
    bf16 = ml_dtypes.bfloat16
    pos_bf = pos.astype(bf16)
    posT_bf = np.ascontiguousarray(pos.T).astype(bf16)
    hot = np.zeros((N, 65), dtype=bf16)
    hot[np.arange(N), batch] = 1
    hot[:, 64] = 1

    def cols(v, nb):
        return np.asarray(v, dtype=np.float32).reshape(nb, 128).T

    prm = np.zeros((128, 32), dtype=np.float32)
    prm[:, 0:4] = cols(inputs["b1"], 4)
    prm[:, 4:8] = cols(inputs["g1"], 4)
    prm[:, 8:12] = cols(inputs["be1"], 4)
    prm[:, 12:16] = cols(inputs["b2"], 4)
    prm[:, 16:20] = cols(inputs["g2"], 4)
    prm[:, 20:24] = cols(inputs["be2"], 4)
    prm[:, 24:26] = cols(inputs["b3"], 2)
    prm[:, 26] = np.float32(np.asarray(inputs["a1"]).reshape(-1)[0])
    prm[:, 27] = np.float32(np.asarray(inputs["a2"]).reshape(-1)[0])
    prm[:, 28] = np.float32(np.log(1.0 / TAU))

    coef = np.array([[-LAM / N, LAM / N, -(1.0 - LAM) / N, (1.0 - LAM) / N,
                      ALPHA / G - BETA / G, -ALPHA / (2 * G), -ALPHA / (2 * G),
                      BETA / G]], dtype=np.float32)

    shared = {
        "W1": np.asarray(inputs["W1"], dtype=np.float32),
        "W2": np.asarray(inputs["W2"], dtype=np.float32),
        "W3": np.asarray(inputs["W3"], dtype=np.float32),
        "prm": prm, "coef": coef,
    }
    in_maps = []
    for c in range(NC):
        sl = slice(c * R, (c + 1) * R)
        m = dict(shared)
        m["za_s"] = za[sl]
        m["zb_s"] = zb[sl]
        m["pos_s"] = pos_bf[sl]
        m["posT_s"] = posT_bf[sl]
        m["hot_s"] = hot[sl]
        in_maps.append(m)

    res = run_bass_kernel_spmd(nc, in_maps, list(range(NC)))
    LAST_RESULTS = res
    out = np.concatenate([res.results[c]["out_s"] for c in range(NC)], axis=0)
    loss = np.float32(res.results[0]["loss"][0, 0])
    return loss, out
